# revision 1
# baseline (speedup 1.0000x reference)
"""AnyPrecisionLinear (4-bit LUT dequant + CSR outliers + bias) on 8 TRN2 cores.

Sharding: 4-way over out_features (O) x 2-way over tokens (B*S).
Core c handles o in [1024*(c%4), +1024), tokens [4096*(c//4), +4096).

Device does all value math:
  - W (bf16) built from lut via one broadcast-copy + GPSIMD local_scatter with
    host-computed slot tables (pure index preprocessing of qweight bits).
  - CSR outlier values converted and scattered on device, added to W.
  - x converted f32->bf16 on ScalarE, transposed via DMA xbar.
  - GEMM on TensorE (bf16, f32 PSUM accum), bias folded in as a K=1 matmul.
Host does only layout/index work: sharding, bit-plane->index repack, sort/slot
tables, CSR indptr parsing + dedup, output concat.
"""

import os
import numpy as np
from contextlib import ExitStack

# Problem constants (hardcoded per harness contract).
B, S, I, O = 4, 2048, 4096, 4096
W_BITS = 4
NT_TOTAL = B * S          # 8192 tokens
N_CORES = 8
O_WAY, N_WAY = 4, 2       # sharding grid
O_SH = O // O_WAY         # 1024 out features per core
N_SH = NT_TOTAL // N_WAY  # 4096 tokens per core
NCHUNK = N_SH // 128      # 32 token chunks per core
OT = O_SH // 128          # 8 o-tiles of 128 rows per core
CH = 1024                 # i-chunk size for local_scatter (num_elems limit 2046)
NCH = I // CH             # 4
IC = I // 128             # 32 i-blocks for the GEMM

_GRAPH_CACHE = {}
PE_FRAC = 0.5
RING_MODE = "single"

import ml_dtypes

_EYE = np.eye(128, dtype=ml_dtypes.bfloat16)


def _host_indices(qweight):
    """bit-planes -> 4-bit index array [O, I] (uint8). Pure bit relayout."""
    shifts = np.arange(32, dtype=np.int32)
    # bits[b, o, w, s] = bit s of qweight[b, o, w]
    bits = ((qweight[:, :, :, None] >> shifts) & 1).astype(np.uint8)
    planew = (1 << (W_BITS - 1 - np.arange(W_BITS))).astype(np.uint8)
    idx = (bits * planew[:, None, None, None]).sum(axis=0, dtype=np.int32)
    return idx.reshape(O, I).astype(np.uint8)


def _scatter_tables(idx, rows, cols, vals):
    """Merged dequant+CSR local_scatter tables.

    Device scatters DELTA values (lut[v]-lut[0]; csr positions get
    cv+lut[v]-lut[0]) and then adds lut[0] per partition, so unwritten
    (value-0, non-csr) positions come out right with no predication.

    Table layout per (row o, chunk ch): [CSE csr slots | 16*S dequant slots];
    dequant slot 16*s+v = position of s-th occurrence of v (v>=1, csr
    positions excluded).  Returns:
      tbl   [O, NCH, CSE+NI] int16   scatter indices (-1 pad)
      vsel  [O, NCH, CSE]    int16   lut index of each csr slot (0 pad)
      cvals [O, NCH, CSE]    f32     csr value of each slot (0 pad)
      NI, CSE
    """
    # ---- CSR -> dedup'd COO ----
    nnz = cols.shape[0]
    row_ids = (np.searchsorted(rows, np.arange(nnz), side="right") - 1).astype(np.int64)
    key = row_ids * I + cols.astype(np.int64)
    uk, inv = np.unique(key, return_inverse=True)
    v2 = np.zeros(len(uk), np.float64)
    np.add.at(v2, inv, vals.astype(np.float64))
    r2 = uk // I
    c2 = uk % I
    ch2 = c2 // CH
    cl2 = (c2 % CH).astype(np.int16)
    grp = r2 * NCH + ch2  # ascending (uk sorted)
    _, gstart, gcount = np.unique(grp, return_index=True, return_counts=True)
    CSE = int(gcount.max())
    CSE += CSE % 2
    CSE = max(CSE, 2)
    rank = np.arange(len(uk)) - np.repeat(gstart, gcount)

    is_csr = np.zeros((O, NCH, CH), bool)
    is_csr[r2, ch2, cl2] = True

    # ---- dequant slots (v>=1, non-csr) ----
    idx4 = idx.reshape(O, NCH, CH).astype(np.int16)
    order = np.argsort(idx4, axis=-1, kind="stable").astype(np.int16)
    sortedv = np.take_along_axis(idx4, order.astype(np.int64), axis=-1)
    cnt = np.zeros((O, NCH, 16), np.int32)
    for v in range(16):
        cnt[:, :, v] = (idx4 == v).sum(-1)
    S = int(cnt[:, :, 1:].max())
    NI = 16 * S
    if NI % 2:
        NI += 16
    cstart = np.concatenate(
        [np.zeros((O, NCH, 1), np.int32), np.cumsum(cnt, -1)[:, :, :-1]], -1
    )
    srank = np.arange(CH)[None, None, :] - np.take_along_axis(
        cstart, sortedv.astype(np.int64), axis=-1
    )
    sorted_is_csr = np.take_along_axis(is_csr, order.astype(np.int64), axis=-1)
    keep = (sortedv > 0) & ~sorted_is_csr
    W = CSE + NI
    scratch = NI + 16 * (CH // 16 + 2)
    tbl = np.full((O, NCH, CSE + scratch), -1, np.int16)
    flat = (CSE + 16 * srank + sortedv).astype(np.int64)
    np.put_along_axis(
        tbl, np.where(keep, flat, tbl.shape[-1] - 1),
        np.where(keep, order, -1), axis=-1,
    )
    tbl = tbl[:, :, :W].copy()
    # ---- csr slots ----
    vsel = np.zeros((O, NCH, CSE), np.int16)
    cvals = np.zeros((O, NCH, CSE), np.float32)
    csr_tbl = np.full((O, NCH, CSE), -1, np.int16)
    csr_tbl[r2, ch2, rank] = cl2
    vsel[r2, ch2, rank] = idx4[r2, ch2, cl2.astype(np.int64)]
    cvals[r2, ch2, rank] = v2.astype(np.float32)
    tbl[:, :, :CSE] = csr_tbl
    return tbl, vsel, cvals, NI, CSE


def _build_graph(NI, CS, pe_frac=0.5, parts="dwxg", ring_mode="split"):
    # parts: "d" dequant-scatter, "w" W-transposes, "x" x pipeline, "g" GEMM+y
    import concourse.bass as bass
    import concourse.bacc as bacc
    import concourse.tile as tile
    from concourse import mybir

    dt = mybir.dt
    nc = bacc.Bacc("TRN2", target_bir_lowering=False, debug=False)

    CSE = CS
    W = CSE + NI
    x_d = nc.dram_tensor("x", [NCHUNK, 128, I], dt.float32, kind="ExternalInput")
    lut_d = nc.dram_tensor("lut", [OT, 128, 16], dt.float32, kind="ExternalInput")
    qid_d = nc.dram_tensor("qidx", [OT, 128, NCH * W], dt.int16, kind="ExternalInput")
    vse_d = nc.dram_tensor("vsel", [OT, 128, NCH * CSE], dt.int16, kind="ExternalInput")
    cva_d = nc.dram_tensor("cvals", [OT, 128, NCH * CSE], dt.float32, kind="ExternalInput")
    bias_d = nc.dram_tensor("bias", [1, O_SH], dt.float32, kind="ExternalInput")
    eye_d = nc.dram_tensor("eye", [128, 128], dt.bfloat16, kind="ExternalInput")
    y_d = nc.dram_tensor("y", [NCHUNK, 128, O_SH], dt.float32, kind="ExternalOutput")
    # ic-blocks transposed on the PE (rest go through the DMA xbar rings)
    pe_ics = set(range(IC))
    n_pe = int(IC * pe_frac)
    pe_ics = set(range(0, IC, max(1, IC // max(n_pe, 1))))  # spread evenly
    while len(pe_ics) > n_pe:
        pe_ics.pop()

    ld_eng = nc.sync
    big_eng = nc.sync
    tr_eng = nc.sync if ring_mode == "single" else nc.scalar
    with tile.TileContext(nc) as tc, ExitStack() as ctx:
        const = ctx.enter_context(tc.tile_pool(name="const", bufs=1))
        wpool = ctx.enter_context(tc.tile_pool(name="w", bufs=2))
        spool = ctx.enter_context(tc.tile_pool(name="scat", bufs=2))
        qpool = ctx.enter_context(tc.tile_pool(name="qp", bufs=1))
        xfpool = ctx.enter_context(tc.tile_pool(name="xf", bufs=2))
        xpool = ctx.enter_context(tc.tile_pool(name="x", bufs=2))
        yopool = ctx.enter_context(tc.tile_pool(name="yo", bufs=2))
        psum = ctx.enter_context(
            tc.tile_pool(name="ps", bufs=2, space=bass.MemorySpace.PSUM)
        )
        pst = ctx.enter_context(
            tc.tile_pool(name="pst", bufs=4, space=bass.MemorySpace.PSUM)
        )

        # Resident transposed weights: WT[p, 1024*ic + ol] = W[ol, 128*ic + p]
        WT = const.tile([128, IC * O_SH], dt.bfloat16)

        ones = const.tile([1, 128], dt.bfloat16)
        nc.vector.memset(ones[:, :], 1.0)
        browf = const.tile([1, O_SH], dt.float32)
        nc.sync.dma_start(browf[:, :], bias_d[:, :])
        brow = const.tile([1, O_SH], dt.bfloat16)
        nc.scalar.copy(brow[:, :], browf[:, :])
        eye = const.tile([128, 128], dt.bfloat16)
        ld_eng.dma_start(eye[:, :], eye_d[:, :])

        if "d" not in parts:
            nc.vector.memset(WT[:, 0:512], 0.125)
        # ---- dequant + CSR (merged single scatter per chunk) ----
        for t in range(OT if "d" in parts else 0):
            lutf = spool.tile([128, 16], dt.float32, tag="lutf")
            ld_eng.dma_start(lutf[:, :], lut_d[t])
            lutb = spool.tile([128, 16], dt.bfloat16, tag="lutb")
            nc.vector.tensor_copy(lutb[:, :], lutf[:, :])
            lutdf = spool.tile([128, 16], dt.float32, tag="lutdf")
            nc.vector.tensor_scalar(
                lutdf[:, :], lutf[:, :], lutf[:, 0:1], None,
                mybir.AluOpType.subtract,
            )
            lutd = spool.tile([128, 16], dt.bfloat16, tag="lutd")
            nc.vector.tensor_copy(lutd[:, :], lutdf[:, :])
            # delta-lut pattern repeated SLOTS times (log-doubling copies)
            pat = spool.tile([128, NI], dt.bfloat16, tag="pat")
            nc.vector.tensor_copy(pat[:, 0:16], lutd[:, :])
            sz = 16
            while sz < NI:
                cp = min(sz, NI - sz)
                nc.vector.tensor_copy(pat[:, sz : sz + cp], pat[:, 0:cp])
                sz += cp
            # csr combined deltas: cv + lutd[v]
            vsl = spool.tile([128, NCH * CSE], dt.int16, tag="vsl")
            ld_eng.dma_start(vsl[:, :], vse_d[t])
            cvf = spool.tile([128, NCH * CSE], dt.float32, tag="cvf")
            ld_eng.dma_start(cvf[:, :], cva_d[t])
            comb = spool.tile([128, NCH * CSE], dt.bfloat16, tag="comb")
            nc.vector.tensor_copy(comb[:, :], cvf[:, :])
            tmp = spool.tile([128, NCH * CSE], dt.bfloat16, tag="tmp")
            for v in range(1, 16):
                nc.vector.tensor_scalar(
                    tmp[:, :], vsl[:, :], float(v), lutdf[:, v : v + 1],
                    mybir.AluOpType.is_equal, mybir.AluOpType.mult,
                )
                nc.vector.tensor_add(comb[:, :], comb[:, :], tmp[:, :])
            qix = qpool.tile([128, NCH * W], dt.int16, tag="qix")
            ld_eng.dma_start(qix[:, :], qid_d[t])
            Wt = wpool.tile([128, I], dt.bfloat16, tag="W")
            for ch in range(NCH):
                sl = slice(ch * CH, (ch + 1) * CH)
                data = spool.tile([128, W], dt.bfloat16, tag="data")
                nc.vector.tensor_copy(
                    data[:, 0:CSE], comb[:, ch * CSE : (ch + 1) * CSE]
                )
                nc.vector.tensor_copy(data[:, CSE:], pat[:, :])
                nc.gpsimd.local_scatter(
                    Wt[:, sl], data[:, :], qix[:, ch * W : (ch + 1) * W],
                    channels=128, num_elems=CH, num_idxs=W,
                )
                nc.vector.tensor_scalar(
                    Wt[:, sl], Wt[:, sl], lutf[:, 0:1], None,
                    mybir.AluOpType.add,
                )
            if "w" in parts:
                for ic in range(IC):
                    eng = tr_eng
                    eng.dma_start_transpose(
                        WT[:, O_SH * ic + 128 * t : O_SH * ic + 128 * (t + 1)],
                        Wt[:, 128 * ic : 128 * (ic + 1)],
                    )

        # ---- GEMM ----
        for n in range(NCHUNK if ("x" in parts or "g" in parts) else 0):
            xT = xpool.tile([128, I], dt.bfloat16, tag="xT")
            if "x" in parts:
                xf = xfpool.tile([128, I], dt.float32, tag="xf")
                big_eng.dma_start(xf[:, :], x_d[n])
                xb = xpool.tile([128, I], dt.bfloat16, tag="xb")
                nc.scalar.copy(xb[:, :], xf[:, :])
                for ic in range(IC):
                    src = xb[:, 128 * ic : 128 * (ic + 1)]
                    dst = xT[:, 128 * ic : 128 * (ic + 1)]
                    if ic in pe_ics:
                        pt = pst.tile([128, 128], dt.bfloat16, tag="pt")
                        nc.tensor.transpose(pt[:, :], src, eye[:, :])
                        nc.vector.tensor_copy(dst, pt[:, :])
                    else:
                        eng = tr_eng
                        eng.dma_start_transpose(dst, src)
            elif n == 0:
                nc.vector.memset(xT[:, :], 0.25)
            for blk in range(O_SH // 512 if "g" in parts else 0):
                ps = psum.tile([128, 512], dt.float32, tag="ps")
                nc.tensor.matmul(
                    ps[:, :], ones[:, :], brow[:, 512 * blk : 512 * (blk + 1)],
                    start=True, stop=False,
                )
                for ic in range(IC):
                    nc.tensor.matmul(
                        ps[:, :],
                        xT[:, 128 * ic : 128 * (ic + 1)],
                        WT[:, O_SH * ic + 512 * blk : O_SH * ic + 512 * (blk + 1)],
                        start=False, stop=(ic == IC - 1),
                    )
                yo = yopool.tile([128, 512], dt.float32, tag="yo")
                nc.vector.tensor_copy(yo[:, :], ps[:, :])
                big_eng.dma_start(y_d[n][:, 512 * blk : 512 * (blk + 1)], yo[:, :])

    nc.compile()
    return nc


def _prep_inputs(x, qweight, lut, rows, cols, vals, bias):
    x = np.ascontiguousarray(np.asarray(x, dtype=np.float32))
    qweight = np.asarray(qweight, dtype=np.int32)
    lut = np.asarray(lut, dtype=np.float32)
    rows = np.asarray(rows, dtype=np.int64)
    cols = np.asarray(cols, dtype=np.int64)
    vals = np.asarray(vals, dtype=np.float32)
    bias = np.asarray(bias, dtype=np.float32)

    idx = _host_indices(qweight)
    tbl, vsel, cvals, NI, CSE = _scatter_tables(idx, rows, cols, vals)
    W = CSE + NI

    x2 = x.reshape(NT_TOTAL, I)
    in_maps = []
    for c in range(N_CORES):
        oq, nh = c % O_WAY, c // O_WAY
        osl = slice(O_SH * oq, O_SH * (oq + 1))
        nsl = slice(N_SH * nh, N_SH * (nh + 1))
        in_maps.append(
            {
                "x": np.ascontiguousarray(x2[nsl].reshape(NCHUNK, 128, I)),
                "lut": np.ascontiguousarray(lut[osl].reshape(OT, 128, 16)),
                # chunk-major per o-tile row: [OT, 128, NCH*W]
                "qidx": np.ascontiguousarray(
                    tbl[osl].reshape(OT, 128, NCH * W)
                ),
                "vsel": np.ascontiguousarray(
                    vsel[osl].reshape(OT, 128, NCH * CSE)
                ),
                "cvals": np.ascontiguousarray(
                    cvals[osl].reshape(OT, 128, NCH * CSE)
                ),
                "bias": np.ascontiguousarray(bias[osl].reshape(1, O_SH)),
                "eye": _EYE,
            }
        )
    return in_maps, NI, CSE


def _run(inputs, trace=False, trace_kwargs=None):
    from concourse.bass_utils import run_bass_kernel_spmd

    in_maps, NI, CS = _prep_inputs(**inputs)

    key = (NI, CS, PE_FRAC, RING_MODE)
    if key not in _GRAPH_CACHE:
        _GRAPH_CACHE[key] = _build_graph(
            NI, CS, pe_frac=PE_FRAC, ring_mode=RING_MODE
        )
    nc = _GRAPH_CACHE[key]

    res = run_bass_kernel_spmd(
        nc, in_maps, core_ids=list(range(N_CORES)),
        trace=trace, **(trace_kwargs or {}),
    )
    out = np.empty((NT_TOTAL, O), np.float32)
    for c in range(N_CORES):
        oq, nh = c % O_WAY, c // O_WAY
        yc = res.results[c]["y"].reshape(N_SH, O_SH)
        out[N_SH * nh : N_SH * (nh + 1), O_SH * oq : O_SH * (oq + 1)] = yc
    return out.reshape(B, S, O), res


def kernel(x, qweight, lut, rows, cols, vals, bias):
    out, _ = _run(dict(x=x, qweight=qweight, lut=lut, rows=rows,
                       cols=cols, vals=vals, bias=bias))
    return out



# revision 8
# speedup vs baseline: 2.3174x; 2.3174x over previous
"""AnyPrecisionLinear (4-bit LUT dequant + CSR outliers + bias) on 8 TRN2 cores.

Sharding: 8-way over out_features (O); tokens replicated.
Core c handles o in [512*c, 512*(c+1)), all 8192 tokens.

Device does all value math:
  - W rows built from lut via GPSIMD local_scatter of full LUT values
    (host precomputes pure index slot tables from qweight bits).
  - CSR outlier values: lut part selected by one tiny local_scatter from the
    replicated-lut pattern, added to DMA'd CSR values on DVE, merged into the
    same per-chunk scatter.
  - W transposed on the PE (is_transpose matmul), 4 blocks per PSUM tile.
  - GEMM on TensorE (bf16, f32 PSUM accum); bias added on DVE at copy-out
    from a partition_broadcast bias row.
Host does only layout/index work: sharding, bit-plane->index repack, slot
tables, CSR indptr parsing + dedup, x transpose to [chunk, i, token] layout
(+ f32->bf16 rounding), output concat.
"""

import numpy as np
from contextlib import ExitStack

import ml_dtypes

# Problem constants (hardcoded per harness contract).
B, S, I, O = 4, 2048, 4096, 4096
W_BITS = 4
NT = B * S                # 8192 tokens
N_CORES = 8
O_SH = O // N_CORES       # 512 out features per core
OT = O_SH // 128          # 4 o-tiles of 128 rows per core
NCHUNK = NT // 128        # 64 token chunks
IC = I // 128             # 32 i-blocks
CH = 1024                 # i-chunk size for local_scatter
NCH = I // CH             # 4

XB = 2                    # x chunks per DMA
YB = 4                    # y chunks per DMA store

_GRAPH_CACHE = {}

_EYE = np.eye(128, dtype=ml_dtypes.bfloat16)


def _host_indices(qweight):
    """bit-planes -> 4-bit index array [O, I] (uint8). Pure bit relayout."""
    shifts = np.arange(32, dtype=np.int32)
    bits = ((qweight[:, :, :, None] >> shifts) & 1).astype(np.uint8)
    planew = (1 << (W_BITS - 1 - np.arange(W_BITS))).astype(np.uint8)
    idx = (bits * planew[:, None, None, None]).sum(axis=0, dtype=np.int32)
    return idx.reshape(O, I).astype(np.uint8)


def _scatter_tables(idx, rows, cols, vals):
    """Slot tables for the merged dequant+CSR local_scatter.

    Per o-row the device holds one data strip [4*CSE + NI]:
      [0 : 4*CSE)        comb slots: chunk-major CSR values (cv + lut[v])
      [4*CSE : 4*CSE+NI) pattern slots: slot 16*s+v holds lut[o, v]
    Chunk ch's scatter uses idx table tbl[o, ch] over the whole strip; slots
    belonging to other chunks (or unused) are -1.

    Returns:
      tbl   [O, NCH, W] int16  scatter dest (position in chunk) or -1
      cst   [O, SC16]   int16  tiny-scatter dest (comb slot) for CSR lut part
      cva   [O, NCH*CSE] f32   CSR values (0 pad)
      CSE, NI, SC16
    """
    nnz = cols.shape[0]
    row_ids = (np.searchsorted(rows, np.arange(nnz), side="right") - 1).astype(np.int64)
    key = row_ids * I + cols.astype(np.int64)
    uk, inv = np.unique(key, return_inverse=True)
    v2 = np.zeros(len(uk), np.float64)
    np.add.at(v2, inv, vals.astype(np.float64))
    r2 = uk // I
    c2 = uk % I
    ch2 = (c2 // CH).astype(np.int64)
    cl2 = (c2 % CH).astype(np.int16)

    grp = r2 * NCH + ch2                       # ascending (uk sorted)
    _, gstart, gcount = np.unique(grp, return_index=True, return_counts=True)
    CSE = int(gcount.max())
    CSE += CSE % 2
    CSE = max(CSE, 2)
    rank = np.arange(len(uk)) - np.repeat(gstart, gcount)

    is_csr = np.zeros((O, NCH, CH), bool)
    is_csr[r2, ch2, cl2] = True

    # ---- dequant slots: all 16 values, csr positions excluded ----
    idx4 = idx.reshape(O, NCH, CH).astype(np.int16)
    idxm = np.where(is_csr, np.int16(16), idx4)          # sentinel sorts last
    order = np.argsort(idxm, axis=-1, kind="stable").astype(np.int16)
    sortedv = np.take_along_axis(idxm, order.astype(np.int64), axis=-1)
    keep = sortedv < 16
    cnt = np.zeros((O, NCH, 16), np.int32)
    for v in range(16):
        cnt[:, :, v] = (idxm == v).sum(-1)
    Smax = int(cnt.max())
    NI = 16 * Smax
    cstart = np.concatenate(
        [np.zeros((O, NCH, 1), np.int32), np.cumsum(cnt, -1)[:, :, :-1]], -1
    )
    srank = np.arange(CH)[None, None, :] - np.take_along_axis(
        cstart, np.minimum(sortedv, 15).astype(np.int64), axis=-1
    )
    W = 4 * CSE + NI
    tbl = np.full((O, NCH, W + 2), -1, np.int16)
    slot = (4 * CSE + 16 * srank + sortedv).astype(np.int64)
    np.put_along_axis(
        tbl, np.where(keep, slot, W + 1),
        np.where(keep, order, -1), axis=-1,
    )
    tbl = tbl[:, :, :W].copy()

    # ---- csr dest slots in the per-chunk tables ----
    comb_slot = (ch2 * CSE + rank).astype(np.int64)
    tbl[r2, ch2, comb_slot] = cl2

    # ---- csr values + tiny-scatter table (lut part of comb) ----
    cva = np.zeros((O, NCH * CSE), np.float32)
    cva[r2, comb_slot] = v2.astype(np.float32)
    vsl = idx4[r2, ch2, cl2.astype(np.int64)]            # lut index per entry
    # occurrence rank of (row, v) among csr entries of that row
    keyrv = r2 * 16 + vsl
    ord2 = np.argsort(keyrv, kind="stable")
    kr_sorted = keyrv[ord2]
    _, g2start, g2count = np.unique(kr_sorted, return_index=True, return_counts=True)
    rank2 = np.empty(len(uk), np.int64)
    rank2[ord2] = np.arange(len(uk)) - np.repeat(g2start, g2count)
    SC = max(int(g2count.max()), 1)
    SC16 = 16 * SC
    cst = np.full((O, SC16), -1, np.int16)
    cst[r2, 16 * rank2 + vsl] = comb_slot.astype(np.int16)
    return tbl, cst, cva, CSE, NI, SC16


def _build_graph(CSE, NI, SC16):
    import concourse.bass as bass
    import concourse.bacc as bacc
    import concourse.tile as tile
    from concourse import mybir

    dt = mybir.dt
    nc = bacc.Bacc("TRN2", target_bir_lowering=False, debug=False)

    WCOL = 4 * CSE + NI
    x_d = nc.dram_tensor("x", [NCHUNK, 128, I], dt.bfloat16, kind="ExternalInput")
    lut_d = nc.dram_tensor("lut", [OT, 128, 16], dt.float32, kind="ExternalInput")
    qid_d = nc.dram_tensor("qidx", [OT, 128, NCH * WCOL], dt.int16, kind="ExternalInput")
    cst_d = nc.dram_tensor("cst", [OT, 128, SC16], dt.int16, kind="ExternalInput")
    cva_d = nc.dram_tensor("cvals", [OT, 128, NCH * CSE], dt.float32, kind="ExternalInput")
    bias_d = nc.dram_tensor("bias", [1, O_SH], dt.float32, kind="ExternalInput")
    eye_d = nc.dram_tensor("eye", [128, 128], dt.bfloat16, kind="ExternalInput")
    y_d = nc.dram_tensor("y", [NCHUNK, 128, O_SH], dt.float32, kind="ExternalOutput")

    IH = I // 2               # i-columns per GEMM phase (A: 0..IH, B: IH..I)
    ICH = IC // 2             # 16 i-blocks per phase
    INS1, INS2 = 30, 45       # A-chunk indices after which ch2/ch3 transposes go

    with tile.TileContext(nc) as tc, ExitStack() as ctx:
        const = ctx.enter_context(tc.tile_pool(name="const", bufs=1))
        dpool = ctx.enter_context(tc.tile_pool(name="dp", bufs=1))
        qpool = ctx.enter_context(tc.tile_pool(name="qp", bufs=3))
        spool = ctx.enter_context(tc.tile_pool(name="sp", bufs=2))
        wpool = ctx.enter_context(tc.tile_pool(name="w", bufs=2))
        xpool = ctx.enter_context(tc.tile_pool(name="x", bufs=2))
        ypool = ctx.enter_context(tc.tile_pool(name="ya", bufs=2))
        y2pool = ctx.enter_context(tc.tile_pool(name="yb", bufs=2))
        psA = ctx.enter_context(
            tc.tile_pool(name="psA", bufs=2, space=bass.MemorySpace.PSUM)
        )
        psB = ctx.enter_context(
            tc.tile_pool(name="psB", bufs=2, space=bass.MemorySpace.PSUM)
        )
        pst = ctx.enter_context(
            tc.tile_pool(name="pst", bufs=2, space=bass.MemorySpace.PSUM)
        )

        # Resident transposed weights: WT[p, 512*ic + 128*t + ol] = W[128*t+ol, 128*ic+p]
        WT = const.tile([128, IC * O_SH], dt.bfloat16)

        eye = const.tile([128, 128], dt.bfloat16)
        nc.sync.dma_start(eye[:, :], eye_d[:, :])
        browp = const.tile([1, O_SH], dt.float32)
        nc.scalar.dma_start(browp[:, :], bias_d[:, :])
        brow = const.tile([128, O_SH], dt.float32)
        nc.gpsimd.partition_broadcast(brow[:, :], browp[:, :])

        # ---- per-tile preps: pattern + CSR comb values ----
        datas, qixs = [], []
        for t in range(OT):
            lutf = spool.tile([128, 16], dt.float32, tag="lutf")
            nc.scalar.dma_start(lutf[:, :], lut_d[t])
            data = dpool.tile([128, WCOL], dt.bfloat16, tag=f"data{t}")
            nc.vector.tensor_copy(data[:, 4 * CSE : 4 * CSE + 16], lutf[:, :])
            sz = 16
            while sz < NI:
                cp = min(sz, NI - sz)
                nc.vector.tensor_copy(
                    data[:, 4 * CSE + sz : 4 * CSE + sz + cp],
                    data[:, 4 * CSE : 4 * CSE + cp],
                )
                sz += cp
            cstt = spool.tile([128, SC16], dt.int16, tag="cst")
            nc.scalar.dma_start(cstt[:, :], cst_d[t])
            nc.gpsimd.local_scatter(
                data[:, 0 : 4 * CSE], data[:, 4 * CSE : 4 * CSE + SC16],
                cstt[:, :], channels=128, num_elems=4 * CSE, num_idxs=SC16,
            )
            cvf = spool.tile([128, NCH * CSE], dt.float32, tag="cvf")
            nc.scalar.dma_start(cvf[:, :], cva_d[t])
            cvb = spool.tile([128, NCH * CSE], dt.bfloat16, tag="cvb")
            nc.vector.tensor_copy(cvb[:, :], cvf[:, :])
            nc.vector.tensor_add(
                data[:, 0 : 4 * CSE], data[:, 0 : 4 * CSE], cvb[:, :]
            )
            datas.append(data)

        def dequant_round(ch):
            for t in range(OT):
                qix = qpool.tile([128, WCOL], dt.int16, tag="qix")
                nc.scalar.dma_start(
                    qix[:, :],
                    qid_d[t][:, ch * WCOL : (ch + 1) * WCOL],
                )
                wq = wpool.tile([128, CH], dt.bfloat16, tag="wq")
                nc.gpsimd.local_scatter(
                    wq[:, :], datas[t][:, :], qix[:, :],
                    channels=128, num_elems=CH, num_idxs=WCOL,
                )
                for g in range(2):
                    pt = pst.tile([128, 512], dt.bfloat16, tag="pt")
                    for k in range(4):
                        l = 4 * g + k
                        nc.tensor.transpose(
                            pt[:, 128 * k : 128 * (k + 1)],
                            wq[:, 128 * l : 128 * (l + 1)],
                            eye[:, :],
                        )
                    ic0 = 8 * ch + 4 * g
                    dst = WT[:, :].rearrange("p (ic o) -> p ic o", o=O_SH)[
                        :, ic0 : ic0 + 4, 128 * t : 128 * (t + 1)
                    ]
                    nc.vector.tensor_copy(
                        dst, pt[:, :].rearrange("p (a b) -> p a b", b=128)
                    )

        dequant_round(0)
        dequant_round(1)

        # ---- GEMM phase A: i-blocks 0..15, plain store ----
        yo = None
        for n in range(NCHUNK):
            if n % XB == 0:
                xT = xpool.tile([128, XB, IH], dt.bfloat16, tag="xT")
                nc.sync.dma_start(
                    xT[:, :, :],
                    x_d[n : n + XB][:, :, 0:IH].rearrange("a b c -> b a c"),
                )
            if n % YB == 0:
                yo = ypool.tile([128, YB, O_SH], dt.float32, tag="yo")
            ps = psA.tile([128, O_SH], dt.float32, tag="psa")
            for ic in range(ICH):
                nc.tensor.matmul(
                    ps[:, :],
                    xT[:, n % XB, 128 * ic : 128 * (ic + 1)],
                    WT[:, O_SH * ic : O_SH * (ic + 1)],
                    start=(ic == 0), stop=(ic == ICH - 1),
                )
            nc.vector.tensor_add(yo[:, n % YB, :], ps[:, :], brow[:, :])
            if n % YB == YB - 1:
                nc.scalar.dma_start(
                    y_d[n - YB + 1 : n + 1].rearrange("a b c -> b a c"),
                    yo[:, :, :],
                )
            if n == INS1:
                dequant_round(2)
            if n == INS2:
                dequant_round(3)

        # ---- GEMM phase B: i-blocks 16..31, accumulate into DRAM ----
        for n in range(NCHUNK):
            if n % XB == 0:
                xT = xpool.tile([128, XB, IH], dt.bfloat16, tag="xT")
                nc.sync.dma_start(
                    xT[:, :, :],
                    x_d[n : n + XB][:, :, IH:I].rearrange("a b c -> b a c"),
                )
            if n % YB == 0:
                yo = y2pool.tile([128, YB, O_SH], dt.float32, tag="yo2")
            ps = psB.tile([128, O_SH], dt.float32, tag="psb")
            for ic in range(ICH):
                nc.tensor.matmul(
                    ps[:, :],
                    xT[:, n % XB, 128 * ic : 128 * (ic + 1)],
                    WT[:, O_SH * (ICH + ic) : O_SH * (ICH + ic + 1)],
                    start=(ic == 0), stop=(ic == ICH - 1),
                )
            nc.vector.tensor_copy(yo[:, n % YB, :], ps[:, :])
            if n % YB == YB - 1:
                nc.gpsimd.dma_start(
                    y_d[n - YB + 1 : n + 1].rearrange("a b c -> b a c"),
                    yo[:, :, :],
                    accum_op=mybir.AluOpType.add,
                )

    nc.compile()
    return nc


def _prep_inputs(x, qweight, lut, rows, cols, vals, bias):
    x = np.asarray(x, dtype=np.float32)
    qweight = np.asarray(qweight, dtype=np.int32)
    lut = np.asarray(lut, dtype=np.float32)
    rows = np.asarray(rows, dtype=np.int64)
    cols = np.asarray(cols, dtype=np.int64)
    vals = np.asarray(vals, dtype=np.float32)
    bias = np.asarray(bias, dtype=np.float32)

    idx = _host_indices(qweight)
    tbl, cst, cva, CSE, NI, SC16 = _scatter_tables(idx, rows, cols, vals)

    # x -> [chunk, i, token] bf16 (pure relayout + dtype rounding)
    xt = np.ascontiguousarray(
        x.reshape(NCHUNK, 128, IC, 128).transpose(0, 3, 2, 1)
    ).reshape(NCHUNK, 128, I).astype(ml_dtypes.bfloat16)

    in_maps = []
    for c in range(N_CORES):
        osl = slice(O_SH * c, O_SH * (c + 1))
        in_maps.append(
            {
                "x": xt,
                "lut": np.ascontiguousarray(lut[osl].reshape(OT, 128, 16)),
                "qidx": np.ascontiguousarray(
                    tbl[osl].reshape(OT, 128, -1)
                ),
                "cst": np.ascontiguousarray(cst[osl].reshape(OT, 128, SC16)),
                "cvals": np.ascontiguousarray(cva[osl].reshape(OT, 128, -1)),
                "bias": np.ascontiguousarray(bias[osl].reshape(1, O_SH)),
                "eye": _EYE,
            }
        )
    return in_maps, CSE, NI, SC16


def _run(inputs, trace=False, trace_kwargs=None):
    from concourse.bass_utils import run_bass_kernel_spmd

    in_maps, CSE, NI, SC16 = _prep_inputs(**inputs)

    key = (CSE, NI, SC16)
    if key not in _GRAPH_CACHE:
        _GRAPH_CACHE[key] = _build_graph(CSE, NI, SC16)
    nc = _GRAPH_CACHE[key]

    res = run_bass_kernel_spmd(
        nc, in_maps, core_ids=list(range(N_CORES)),
        trace=trace, **(trace_kwargs or {}),
    )
    out = np.empty((NT, O), np.float32)
    for c in range(N_CORES):
        yc = res.results[c]["y"].reshape(NT, O_SH)
        out[:, O_SH * c : O_SH * (c + 1)] = yc
    return out.reshape(B, S, O), res


def kernel(x, qweight, lut, rows, cols, vals, bias):
    out, _ = _run(dict(x=x, qweight=qweight, lut=lut, rows=rows,
                       cols=cols, vals=vals, bias=bias))
    return out


# revision 17
# speedup vs baseline: 2.6966x; 1.1637x over previous
"""AnyPrecisionLinear (4-bit LUT dequant + CSR outliers + bias) on 8 TRN2 cores.

Sharding: 8-way over out_features (O); tokens replicated.
Core c handles o in [512*c, 512*(c+1)), all 8192 tokens.

Device does all value math:
  - W rows built from lut via GPSIMD local_scatter of full LUT values
    (host precomputes pure index slot tables from qweight bits).
  - CSR outlier values: lut part selected by one tiny local_scatter from the
    replicated-lut pattern, added to DMA'd CSR values on DVE, merged into the
    same per-chunk scatter.
  - W transposed on the PE (is_transpose matmul), 4 blocks per PSUM tile.
  - GEMM on TensorE (bf16, f32 PSUM accum); bias added on DVE at copy-out
    from a partition_broadcast bias row.
Host does only layout/index work: sharding, bit-plane->index repack, slot
tables, CSR indptr parsing + dedup, x transpose to [chunk, i, token] layout
(+ f32->bf16 rounding), output concat.
"""

import numpy as np
from contextlib import ExitStack

import ml_dtypes

# Problem constants (hardcoded per harness contract).
B, S, I, O = 4, 2048, 4096, 4096
W_BITS = 4
NT = B * S                # 8192 tokens
N_CORES = 8
O_SH = O // N_CORES       # 512 out features per core
OT = O_SH // 128          # 4 o-tiles of 128 rows per core
NCHUNK = NT // 128        # 64 token chunks
IC = I // 128             # 32 i-blocks
CH = 1024                 # i-chunk size for local_scatter
NCH = I // CH             # 4

XB = 2                    # x chunks per DMA
YB = 4                    # y chunks per DMA store

_GRAPH_CACHE = {}

_EYE = np.eye(128, dtype=ml_dtypes.bfloat16)


def _host_indices(qweight):
    """bit-planes -> 4-bit index array [O, I] (uint8). Pure bit relayout."""
    shifts = np.arange(32, dtype=np.int32)
    bits = ((qweight[:, :, :, None] >> shifts) & 1).astype(np.uint8)
    planew = (1 << (W_BITS - 1 - np.arange(W_BITS))).astype(np.uint8)
    idx = (bits * planew[:, None, None, None]).sum(axis=0, dtype=np.int32)
    return idx.reshape(O, I).astype(np.uint8)


def _scatter_tables(idx, rows, cols, vals):
    """Slot tables for the merged dequant+CSR local_scatter.

    Per o-row the device holds one data strip [4*CSE + NI]:
      [0 : 4*CSE)        comb slots: chunk-major CSR values (cv + lut[v])
      [4*CSE : 4*CSE+NI) pattern slots: slot 16*s+v holds lut[o, v]
    Chunk ch's scatter uses idx table tbl[o, ch] over the whole strip; slots
    belonging to other chunks (or unused) are -1.

    Returns:
      tbl   [O, NCH, W] int16  scatter dest (position in chunk) or -1
      cst   [O, SC16]   int16  tiny-scatter dest (comb slot) for CSR lut part
      cva   [O, NCH*CSE] f32   CSR values (0 pad)
      CSE, NI, SC16
    """
    nnz = cols.shape[0]
    row_ids = (np.searchsorted(rows, np.arange(nnz), side="right") - 1).astype(np.int64)
    key = row_ids * I + cols.astype(np.int64)
    uk, inv = np.unique(key, return_inverse=True)
    v2 = np.zeros(len(uk), np.float64)
    np.add.at(v2, inv, vals.astype(np.float64))
    r2 = uk // I
    c2 = uk % I
    ch2 = (c2 // CH).astype(np.int64)
    cl2 = (c2 % CH).astype(np.int16)

    grp = r2 * NCH + ch2                       # ascending (uk sorted)
    _, gstart, gcount = np.unique(grp, return_index=True, return_counts=True)
    CSE = int(gcount.max())
    CSE += CSE % 2
    CSE = max(CSE, 2)
    rank = np.arange(len(uk)) - np.repeat(gstart, gcount)

    is_csr = np.zeros((O, NCH, CH), bool)
    is_csr[r2, ch2, cl2] = True

    # ---- dequant slots: all 16 values, csr positions excluded ----
    idx4 = idx.reshape(O, NCH, CH).astype(np.int16)
    idxm = np.where(is_csr, np.int16(16), idx4)          # sentinel sorts last
    order = np.argsort(idxm, axis=-1, kind="stable").astype(np.int16)
    sortedv = np.take_along_axis(idxm, order.astype(np.int64), axis=-1)
    keep = sortedv < 16
    cnt = np.zeros((O, NCH, 16), np.int32)
    for v in range(16):
        cnt[:, :, v] = (idxm == v).sum(-1)
    Smax = int(cnt.max())
    NI = 16 * Smax
    cstart = np.concatenate(
        [np.zeros((O, NCH, 1), np.int32), np.cumsum(cnt, -1)[:, :, :-1]], -1
    )
    srank = np.arange(CH)[None, None, :] - np.take_along_axis(
        cstart, np.minimum(sortedv, 15).astype(np.int64), axis=-1
    )
    W = 4 * CSE + NI
    tbl = np.full((O, NCH, W + 2), -1, np.int16)
    slot = (4 * CSE + 16 * srank + sortedv).astype(np.int64)
    np.put_along_axis(
        tbl, np.where(keep, slot, W + 1),
        np.where(keep, order, -1), axis=-1,
    )
    tbl = tbl[:, :, :W].copy()

    # ---- csr dest slots in the per-chunk tables ----
    comb_slot = (ch2 * CSE + rank).astype(np.int64)
    tbl[r2, ch2, comb_slot] = cl2

    # ---- csr values + tiny-scatter table (lut part of comb) ----
    cva = np.zeros((O, NCH * CSE), np.float32)
    cva[r2, comb_slot] = v2.astype(np.float32)
    vsl = idx4[r2, ch2, cl2.astype(np.int64)]            # lut index per entry
    # occurrence rank of (row, v) among csr entries of that row
    keyrv = r2 * 16 + vsl
    ord2 = np.argsort(keyrv, kind="stable")
    kr_sorted = keyrv[ord2]
    _, g2start, g2count = np.unique(kr_sorted, return_index=True, return_counts=True)
    rank2 = np.empty(len(uk), np.int64)
    rank2[ord2] = np.arange(len(uk)) - np.repeat(g2start, g2count)
    SC = max(int(g2count.max()), 1)
    SC16 = 16 * SC
    cst = np.full((O, SC16), -1, np.int16)
    cst[r2, 16 * rank2 + vsl] = comb_slot.astype(np.int16)
    return tbl, cst, cva, CSE, NI, SC16


def _build_graph(CSE, NI, SC16):
    import concourse.bass as bass
    import concourse.bacc as bacc
    import concourse.tile as tile
    from concourse import mybir

    dt = mybir.dt
    nc = bacc.Bacc("TRN2", target_bir_lowering=False, debug=False)

    WCOL = 4 * CSE + NI
    x_d = nc.dram_tensor("x", [NCHUNK, 128, I], dt.bfloat16, kind="ExternalInput")
    lut_d = nc.dram_tensor("lut", [OT, 128, 16], dt.float32, kind="ExternalInput")
    qid_d = nc.dram_tensor("qidx", [OT, 128, NCH * WCOL], dt.int16, kind="ExternalInput")
    cst_d = nc.dram_tensor("cst", [OT, 128, SC16], dt.int16, kind="ExternalInput")
    cva_d = nc.dram_tensor("cvals", [OT, 128, NCH * CSE], dt.float32, kind="ExternalInput")
    bias_d = nc.dram_tensor("bias", [1, O_SH], dt.float32, kind="ExternalInput")
    eye_d = nc.dram_tensor("eye", [128, 128], dt.bfloat16, kind="ExternalInput")
    y_d = nc.dram_tensor("y", [NCHUNK, 128, O_SH], dt.float32, kind="ExternalOutput")

    # GEMM phases over i-block ranges; phase k covers scatter-chunk chs PH[k].
    # Matmuls of chunk PAIRS interleave on two PSUM banks to hide the PE
    # array drain (serial fill->drain on one bank costs ~46ns/matmul).
    PH = [(0, 8), (8, 24), (24, 32)]   # [ic0, ic1) per phase
    # transpose-round insertion points: (phase, pair-index) -> ch
    TINS = {(0, 13): 1, (0, 25): 2, (1, 2): 3}

    with tile.TileContext(nc) as tc, ExitStack() as ctx:
        const = ctx.enter_context(tc.tile_pool(name="const", bufs=1))
        dpool = ctx.enter_context(tc.tile_pool(name="dp", bufs=1))
        qpool = ctx.enter_context(tc.tile_pool(name="qp", bufs=3))
        spool = ctx.enter_context(tc.tile_pool(name="sp", bufs=2))
        wpool = ctx.enter_context(tc.tile_pool(name="w", bufs=8))
        xpool = ctx.enter_context(tc.tile_pool(name="x", bufs=3))
        ypool = ctx.enter_context(tc.tile_pool(name="ya", bufs=2))
        y2pool = ctx.enter_context(tc.tile_pool(name="yb", bufs=2))
        psum = ctx.enter_context(
            tc.tile_pool(name="ps", bufs=4, space=bass.MemorySpace.PSUM)
        )
        pst = ctx.enter_context(
            tc.tile_pool(name="pst", bufs=2, space=bass.MemorySpace.PSUM)
        )

        # Resident transposed weights: WT[p, 512*ic + 128*t + ol] = W[128*t+ol, 128*ic+p]
        WT = const.tile([128, IC * O_SH], dt.bfloat16)

        eye = const.tile([128, 128], dt.bfloat16)
        nc.sync.dma_start(eye[:, :], eye_d[:, :])
        browp = const.tile([1, O_SH], dt.float32)
        nc.scalar.dma_start(browp[:, :], bias_d[:, :])
        brow = const.tile([128, O_SH], dt.float32)
        nc.gpsimd.partition_broadcast(brow[:, :], browp[:, :])

        # ---- per-tile preps: pattern + CSR comb values ----
        datas, wqs = [], []

        def dequant_one(ch, t, dma_eng):
            qix = qpool.tile([128, WCOL], dt.int16, tag="qix")
            dma_eng.dma_start(
                qix[:, :], qid_d[t][:, ch * WCOL : (ch + 1) * WCOL]
            )
            wq = wpool.tile([128, CH], dt.bfloat16, tag="wq")
            nc.gpsimd.local_scatter(
                wq[:, :], datas[t][:, :], qix[:, :],
                channels=128, num_elems=CH, num_idxs=WCOL,
            )
            wqs.append(wq)

        for t in range(OT):
            lutf = spool.tile([128, 16], dt.float32, tag="lutf")
            nc.sync.dma_start(lutf[:, :], lut_d[t])
            data = dpool.tile([128, WCOL], dt.bfloat16, tag=f"data{t}")
            nc.vector.tensor_copy(data[:, 4 * CSE : 4 * CSE + 16], lutf[:, :])
            sz = 16
            while sz < NI:
                cp = min(sz, NI - sz)
                nc.vector.tensor_copy(
                    data[:, 4 * CSE + sz : 4 * CSE + sz + cp],
                    data[:, 4 * CSE : 4 * CSE + cp],
                )
                sz += cp
            cstt = spool.tile([128, SC16], dt.int16, tag="cst")
            nc.sync.dma_start(cstt[:, :], cst_d[t])
            nc.gpsimd.local_scatter(
                data[:, 0 : 4 * CSE], data[:, 4 * CSE : 4 * CSE + SC16],
                cstt[:, :], channels=128, num_elems=4 * CSE, num_idxs=SC16,
            )
            cvf = spool.tile([128, NCH * CSE], dt.float32, tag="cvf")
            nc.sync.dma_start(cvf[:, :], cva_d[t])
            cvb = spool.tile([128, NCH * CSE], dt.bfloat16, tag="cvb")
            nc.vector.tensor_copy(cvb[:, :], cvf[:, :])
            nc.vector.tensor_add(
                data[:, 0 : 4 * CSE], data[:, 0 : 4 * CSE], cvb[:, :]
            )
            datas.append(data)
            dequant_one(0, t, nc.sync)

        def transpose_round(ch):
            for t in range(OT):
                wq = wqs[4 * ch + t]
                for g in range(2):
                    pt = pst.tile([128, 512], dt.bfloat16, tag="pt")
                    for k in range(4):
                        l = 4 * g + k
                        nc.tensor.transpose(
                            pt[:, 128 * k : 128 * (k + 1)],
                            wq[:, 128 * l : 128 * (l + 1)],
                            eye[:, :],
                        )
                    ic0 = 8 * ch + 4 * g
                    dst = WT[:, :].rearrange("p (ic o) -> p ic o", o=O_SH)[
                        :, ic0 : ic0 + 4, 128 * t : 128 * (t + 1)
                    ]
                    nc.vector.tensor_copy(
                        dst, pt[:, :].rearrange("p (a b) -> p a b", b=128)
                    )

        # ch1-3 table loads stream on the ACT queue while the GEMM runs.
        for ch in range(1, NCH):
            for t in range(OT):
                dequant_one(ch, t, nc.scalar)
        transpose_round(0)

        # ---- GEMM: 3 phases over i-blocks; chunk pairs share the PE ----
        for ph, (ica, icb) in enumerate(PH):
            c0, c1 = 128 * ica, 128 * icb
            first, last = ph == 0, ph == len(PH) - 1
            for p in range(NCHUNK // 2):
                n0 = 2 * p
                xT = xpool.tile([128, 2, c1 - c0], dt.bfloat16, tag="xT")
                nc.sync.dma_start(
                    xT[:, :, :],
                    x_d[n0 : n0 + 2][:, :, c0:c1].rearrange("a b c -> b a c"),
                )
                if n0 % YB == 0:
                    pool = ypool if first else y2pool
                    yo = pool.tile([128, YB, O_SH], dt.float32, tag="yo")
                ps0 = psum.tile([128, O_SH], dt.float32, tag="ps")
                ps1 = psum.tile([128, O_SH], dt.float32, tag="ps")
                pss = [ps0, ps1]
                for ic in range(ica, icb):
                    for j in range(2):
                        nc.tensor.matmul(
                            pss[j][:, :],
                            xT[:, j, 128 * ic - c0 : 128 * (ic + 1) - c0],
                            WT[:, O_SH * ic : O_SH * (ic + 1)],
                            start=(ic == ica), stop=(ic == icb - 1),
                        )
                for j in range(2):
                    if first:
                        nc.vector.tensor_add(
                            yo[:, (n0 + j) % YB, :], pss[j][:, :], brow[:, :]
                        )
                    else:
                        nc.vector.tensor_copy(yo[:, (n0 + j) % YB, :], pss[j][:, :])
                if (n0 + 1) % YB == YB - 1:
                    ysl = y_d[n0 + 2 - YB : n0 + 2].rearrange("a b c -> b a c")
                    if first:
                        nc.scalar.dma_start(ysl, yo[:, :, :])
                    else:
                        nc.gpsimd.dma_start(
                            ysl, yo[:, :, :], accum_op=mybir.AluOpType.add
                        )
                ch = TINS.get((ph, p))
                if ch is not None:
                    transpose_round(ch)

    nc.compile()
    return nc


def _prep_inputs(x, qweight, lut, rows, cols, vals, bias):
    x = np.asarray(x, dtype=np.float32)
    qweight = np.asarray(qweight, dtype=np.int32)
    lut = np.asarray(lut, dtype=np.float32)
    rows = np.asarray(rows, dtype=np.int64)
    cols = np.asarray(cols, dtype=np.int64)
    vals = np.asarray(vals, dtype=np.float32)
    bias = np.asarray(bias, dtype=np.float32)

    idx = _host_indices(qweight)
    tbl, cst, cva, CSE, NI, SC16 = _scatter_tables(idx, rows, cols, vals)

    # x -> [chunk, i, token] bf16 (pure relayout + dtype rounding)
    xt = np.ascontiguousarray(
        x.reshape(NCHUNK, 128, IC, 128).transpose(0, 3, 2, 1)
    ).reshape(NCHUNK, 128, I).astype(ml_dtypes.bfloat16)

    in_maps = []
    for c in range(N_CORES):
        osl = slice(O_SH * c, O_SH * (c + 1))
        in_maps.append(
            {
                "x": xt,
                "lut": np.ascontiguousarray(lut[osl].reshape(OT, 128, 16)),
                "qidx": np.ascontiguousarray(
                    tbl[osl].reshape(OT, 128, -1)
                ),
                "cst": np.ascontiguousarray(cst[osl].reshape(OT, 128, SC16)),
                "cvals": np.ascontiguousarray(cva[osl].reshape(OT, 128, -1)),
                "bias": np.ascontiguousarray(bias[osl].reshape(1, O_SH)),
                "eye": _EYE,
            }
        )
    return in_maps, CSE, NI, SC16


def _run(inputs, trace=False, trace_kwargs=None):
    from concourse.bass_utils import run_bass_kernel_spmd

    in_maps, CSE, NI, SC16 = _prep_inputs(**inputs)

    key = (CSE, NI, SC16)
    if key not in _GRAPH_CACHE:
        _GRAPH_CACHE[key] = _build_graph(CSE, NI, SC16)
    nc = _GRAPH_CACHE[key]

    res = run_bass_kernel_spmd(
        nc, in_maps, core_ids=list(range(N_CORES)),
        trace=trace, **(trace_kwargs or {}),
    )
    out = np.empty((NT, O), np.float32)
    for c in range(N_CORES):
        yc = res.results[c]["y"].reshape(NT, O_SH)
        out[:, O_SH * c : O_SH * (c + 1)] = yc
    return out.reshape(B, S, O), res


def kernel(x, qweight, lut, rows, cols, vals, bias):
    out, _ = _run(dict(x=x, qweight=qweight, lut=lut, rows=rows,
                       cols=cols, vals=vals, bias=bias))
    return out


# revision 22
# speedup vs baseline: 2.8278x; 1.0487x over previous
"""AnyPrecisionLinear (4-bit LUT dequant + CSR outliers + bias) on 8 TRN2 cores.

Sharding: 8-way over out_features (O); tokens replicated.
Core c handles o in [512*c, 512*(c+1)), all 8192 tokens.

Device does all value math:
  - W rows built from lut via GPSIMD local_scatter of full LUT values
    (host precomputes pure index slot tables from qweight bits).
  - CSR outlier values: lut part selected by one tiny local_scatter from the
    replicated-lut pattern, added to DMA'd CSR values on DVE, merged into the
    same per-chunk scatter.
  - W transposed on the PE (is_transpose matmul), 4 blocks per PSUM tile.
  - GEMM on TensorE (bf16, f32 PSUM accum); bias added on DVE at copy-out
    from a partition_broadcast bias row.
Host does only layout/index work: sharding, bit-plane->index repack, slot
tables, CSR indptr parsing + dedup, x transpose to [chunk, i, token] layout
(+ f32->bf16 rounding), output concat.
"""

import numpy as np
from contextlib import ExitStack

import ml_dtypes

# Problem constants (hardcoded per harness contract).
B, S, I, O = 4, 2048, 4096, 4096
W_BITS = 4
NT = B * S                # 8192 tokens
N_CORES = 8
O_SH = O // N_CORES       # 512 out features per core
OT = O_SH // 128          # 4 o-tiles of 128 rows per core
NCHUNK = NT // 128        # 64 token chunks
IC = I // 128             # 32 i-blocks
CH = 1024                 # i-chunk size for local_scatter
NCH = I // CH             # 4

XB = 2                    # x chunks per DMA
YB = 4                    # y chunks per DMA store

_GRAPH_CACHE = {}

_EYE = np.eye(128, dtype=ml_dtypes.bfloat16)


def _host_indices(qweight):
    """bit-planes -> 4-bit index array [O, I] (uint8). Pure bit relayout."""
    shifts = np.arange(32, dtype=np.int32)
    bits = ((qweight[:, :, :, None] >> shifts) & 1).astype(np.uint8)
    planew = (1 << (W_BITS - 1 - np.arange(W_BITS))).astype(np.uint8)
    idx = (bits * planew[:, None, None, None]).sum(axis=0, dtype=np.int32)
    return idx.reshape(O, I).astype(np.uint8)


def _scatter_tables(idx, rows, cols, vals):
    """Slot tables for the merged dequant+CSR local_scatter.

    Per o-row the device holds one data strip [4*CSE + NI]:
      [0 : 4*CSE)        comb slots: chunk-major CSR values (cv + lut[v])
      [4*CSE : 4*CSE+NI) pattern slots: slot 16*s+v holds lut[o, v]
    Chunk ch's scatter uses idx table tbl[o, ch] over the whole strip; slots
    belonging to other chunks (or unused) are -1.

    Returns:
      tbl   [O, NCH, W] int16  scatter dest (position in chunk) or -1
      cst   [O, SC16]   int16  tiny-scatter dest (comb slot) for CSR lut part
      cva   [O, NCH*CSE] f32   CSR values (0 pad)
      CSE, NI, SC16
    """
    nnz = cols.shape[0]
    row_ids = (np.searchsorted(rows, np.arange(nnz), side="right") - 1).astype(np.int64)
    key = row_ids * I + cols.astype(np.int64)
    uk, inv = np.unique(key, return_inverse=True)
    v2 = np.zeros(len(uk), np.float64)
    np.add.at(v2, inv, vals.astype(np.float64))
    r2 = uk // I
    c2 = uk % I
    ch2 = (c2 // CH).astype(np.int64)
    cl2 = (c2 % CH).astype(np.int16)

    grp = r2 * NCH + ch2                       # ascending (uk sorted)
    _, gstart, gcount = np.unique(grp, return_index=True, return_counts=True)
    CSE = int(gcount.max())
    CSE += CSE % 2
    CSE = max(CSE, 2)
    rank = np.arange(len(uk)) - np.repeat(gstart, gcount)

    is_csr = np.zeros((O, NCH, CH), bool)
    is_csr[r2, ch2, cl2] = True

    # ---- dequant slots: all 16 values, csr positions excluded ----
    idx4 = idx.reshape(O, NCH, CH).astype(np.int16)
    idxm = np.where(is_csr, np.int16(16), idx4)          # sentinel sorts last
    order = np.argsort(idxm, axis=-1, kind="stable").astype(np.int16)
    sortedv = np.take_along_axis(idxm, order.astype(np.int64), axis=-1)
    keep = sortedv < 16
    cnt = np.zeros((O, NCH, 16), np.int32)
    for v in range(16):
        cnt[:, :, v] = (idxm == v).sum(-1)
    Smax = int(cnt.max())
    NI = 16 * Smax
    cstart = np.concatenate(
        [np.zeros((O, NCH, 1), np.int32), np.cumsum(cnt, -1)[:, :, :-1]], -1
    )
    srank = np.arange(CH)[None, None, :] - np.take_along_axis(
        cstart, np.minimum(sortedv, 15).astype(np.int64), axis=-1
    )
    W = 4 * CSE + NI
    tbl = np.full((O, NCH, W + 2), -1, np.int16)
    slot = (4 * CSE + 16 * srank + sortedv).astype(np.int64)
    np.put_along_axis(
        tbl, np.where(keep, slot, W + 1),
        np.where(keep, order, -1), axis=-1,
    )
    tbl = tbl[:, :, :W].copy()

    # ---- csr dest slots in the per-chunk tables ----
    comb_slot = (ch2 * CSE + rank).astype(np.int64)
    tbl[r2, ch2, comb_slot] = cl2

    # ---- csr values + tiny-scatter table (lut part of comb) ----
    cva = np.zeros((O, NCH * CSE), np.float32)
    cva[r2, comb_slot] = v2.astype(np.float32)
    vsl = idx4[r2, ch2, cl2.astype(np.int64)]            # lut index per entry
    # occurrence rank of (row, v) among csr entries of that row
    keyrv = r2 * 16 + vsl
    ord2 = np.argsort(keyrv, kind="stable")
    kr_sorted = keyrv[ord2]
    _, g2start, g2count = np.unique(kr_sorted, return_index=True, return_counts=True)
    rank2 = np.empty(len(uk), np.int64)
    rank2[ord2] = np.arange(len(uk)) - np.repeat(g2start, g2count)
    SC = max(int(g2count.max()), 1)
    SC16 = 16 * SC
    cst = np.full((O, SC16), -1, np.int16)
    cst[r2, 16 * rank2 + vsl] = comb_slot.astype(np.int16)
    return tbl, cst, cva, CSE, NI, SC16


def _build_graph(CSE, NI, SC16):
    import concourse.bass as bass
    import concourse.bacc as bacc
    import concourse.tile as tile
    from concourse import mybir

    dt = mybir.dt
    nc = bacc.Bacc("TRN2", target_bir_lowering=False, debug=False)

    WCOL = 4 * CSE + NI
    x_d = nc.dram_tensor("x", [NCHUNK, 128, I], dt.bfloat16, kind="ExternalInput")
    lut_d = nc.dram_tensor("lut", [OT, 128, 16], dt.float32, kind="ExternalInput")
    qid_d = nc.dram_tensor("qidx", [OT, 128, NCH * WCOL], dt.int16, kind="ExternalInput")
    cst_d = nc.dram_tensor("cst", [OT, 128, SC16], dt.int16, kind="ExternalInput")
    cva_d = nc.dram_tensor("cvals", [OT, 128, NCH * CSE], dt.float32, kind="ExternalInput")
    bias_d = nc.dram_tensor("bias", [1, O_SH], dt.float32, kind="ExternalInput")
    eye_d = nc.dram_tensor("eye", [128, 128], dt.bfloat16, kind="ExternalInput")
    y_d = nc.dram_tensor("y", [NCHUNK, 128, O_SH], dt.float32, kind="ExternalOutput")

    # GEMM phases over i-block ranges; phase k covers scatter-chunk chs PH[k].
    # Matmuls of chunk PAIRS interleave on two PSUM banks to hide the PE
    # array drain (serial fill->drain on one bank costs ~46ns/matmul).
    PH = [(0, 8), (8, 32)]             # [ic0, ic1) per phase
    # transpose-round insertion points: (phase, pair-index) -> ch
    TINS = {(0, 7): 1, (0, 16): 2, (0, 26): 3}

    with tile.TileContext(nc) as tc, ExitStack() as ctx:
        const = ctx.enter_context(tc.tile_pool(name="const", bufs=1))
        dpool = ctx.enter_context(tc.tile_pool(name="dp", bufs=1))
        qpool = ctx.enter_context(tc.tile_pool(name="qp", bufs=3))
        spool = ctx.enter_context(tc.tile_pool(name="sp", bufs=2))
        wpool = ctx.enter_context(tc.tile_pool(name="w", bufs=12))
        xpool = ctx.enter_context(tc.tile_pool(name="x", bufs=3))
        ypool = ctx.enter_context(tc.tile_pool(name="ya", bufs=2))
        y2pool = ctx.enter_context(tc.tile_pool(name="yb", bufs=3))
        psum = ctx.enter_context(
            tc.tile_pool(name="ps", bufs=4, space=bass.MemorySpace.PSUM)
        )
        pst = ctx.enter_context(
            tc.tile_pool(name="pst", bufs=2, space=bass.MemorySpace.PSUM)
        )

        # Resident transposed weights: WT[p, 512*ic + 128*t + ol] = W[128*t+ol, 128*ic+p]
        WT = const.tile([128, IC * O_SH], dt.bfloat16)

        eye = const.tile([128, 128], dt.bfloat16)
        nc.scalar.dma_start(eye[:, :], eye_d[:, :])

        # ---- per-tile preps: pattern + CSR comb values ----
        datas, wqs = [], []

        def dequant_one(ch, t, dma_eng):
            qix = qpool.tile([128, WCOL], dt.int16, tag="qix")
            dma_eng.dma_start(
                qix[:, :], qid_d[t][:, ch * WCOL : (ch + 1) * WCOL]
            )
            wq = wpool.tile([128, CH], dt.bfloat16, tag="wq")
            nc.gpsimd.local_scatter(
                wq[:, :], datas[t][:, :], qix[:, :],
                channels=128, num_elems=CH, num_idxs=WCOL,
            )
            wqs.append(wq)

        for t in range(OT):
            lutf = spool.tile([128, 16], dt.float32, tag="lutf")
            nc.sync.dma_start(lutf[:, :], lut_d[t])
            data = dpool.tile([128, WCOL], dt.bfloat16, tag=f"data{t}")
            nc.vector.tensor_copy(data[:, 4 * CSE : 4 * CSE + 16], lutf[:, :])
            sz = 16
            while sz < NI:
                cp = min(sz, NI - sz)
                nc.vector.tensor_copy(
                    data[:, 4 * CSE + sz : 4 * CSE + sz + cp],
                    data[:, 4 * CSE : 4 * CSE + cp],
                )
                sz += cp
            cstt = spool.tile([128, SC16], dt.int16, tag="cst")
            nc.sync.dma_start(cstt[:, :], cst_d[t])
            nc.gpsimd.local_scatter(
                data[:, 0 : 4 * CSE], data[:, 4 * CSE : 4 * CSE + SC16],
                cstt[:, :], channels=128, num_elems=4 * CSE, num_idxs=SC16,
            )
            cvf = spool.tile([128, NCH * CSE], dt.float32, tag="cvf")
            nc.sync.dma_start(cvf[:, :], cva_d[t])
            cvb = spool.tile([128, NCH * CSE], dt.bfloat16, tag="cvb")
            nc.vector.tensor_copy(cvb[:, :], cvf[:, :])
            nc.vector.tensor_add(
                data[:, 0 : 4 * CSE], data[:, 0 : 4 * CSE], cvb[:, :]
            )
            datas.append(data)
            dequant_one(0, t, nc.sync)

        def transpose_round(ch):
            for t in range(OT):
                wq = wqs[4 * ch + t]
                for g in range(2):
                    pt = pst.tile([128, 512], dt.bfloat16, tag="pt")
                    for k in range(4):
                        l = 4 * g + k
                        nc.tensor.transpose(
                            pt[:, 128 * k : 128 * (k + 1)],
                            wq[:, 128 * l : 128 * (l + 1)],
                            eye[:, :],
                        )
                    ic0 = 8 * ch + 4 * g
                    dst = WT[:, :].rearrange("p (ic o) -> p ic o", o=O_SH)[
                        :, ic0 : ic0 + 4, 128 * t : 128 * (t + 1)
                    ]
                    nc.vector.tensor_copy(
                        dst, pt[:, :].rearrange("p (a b) -> p a b", b=128)
                    )

        browp = const.tile([1, O_SH], dt.float32)
        nc.scalar.dma_start(browp[:, :], bias_d[:, :])
        brow = const.tile([128, O_SH], dt.float32)
        nc.gpsimd.partition_broadcast(brow[:, :], browp[:, :])

        # ch1-3 table loads stream on the ACT queue while the GEMM runs.
        for ch in range(1, NCH):
            for t in range(OT):
                dequant_one(ch, t, nc.scalar)
        transpose_round(0)

        # ---- GEMM: 3 phases over i-blocks; chunk pairs share the PE ----
        for ph, (ica, icb) in enumerate(PH):
            c0, c1 = 128 * ica, 128 * icb
            first, last = ph == 0, ph == len(PH) - 1
            for p in range(NCHUNK // 2):
                n0 = 2 * p
                xT = xpool.tile([128, 2, c1 - c0], dt.bfloat16, tag="xT")
                nc.sync.dma_start(
                    xT[:, :, :],
                    x_d[n0 : n0 + 2][:, :, c0:c1].rearrange("a b c -> b a c"),
                )
                if n0 % YB == 0:
                    pool = ypool if first else y2pool
                    yo = pool.tile([128, YB, O_SH], dt.float32, tag="yo")
                ps0 = psum.tile([128, O_SH], dt.float32, tag="ps")
                ps1 = psum.tile([128, O_SH], dt.float32, tag="ps")
                pss = [ps0, ps1]
                for ic in range(ica, icb):
                    for j in range(2):
                        nc.tensor.matmul(
                            pss[j][:, :],
                            xT[:, j, 128 * ic - c0 : 128 * (ic + 1) - c0],
                            WT[:, O_SH * ic : O_SH * (ic + 1)],
                            start=(ic == ica), stop=(ic == icb - 1),
                        )
                for j in range(2):
                    if first:
                        nc.vector.tensor_add(
                            yo[:, (n0 + j) % YB, :], pss[j][:, :], brow[:, :]
                        )
                    else:
                        nc.vector.tensor_copy(yo[:, (n0 + j) % YB, :], pss[j][:, :])
                if (n0 + 1) % YB == YB - 1:
                    ysl = y_d[n0 + 2 - YB : n0 + 2].rearrange("a b c -> b a c")
                    if first:
                        nc.scalar.dma_start(ysl, yo[:, :, :])
                    else:
                        nc.gpsimd.dma_start(
                            ysl, yo[:, :, :], accum_op=mybir.AluOpType.add
                        )
                ch = TINS.get((ph, p))
                if ch is not None:
                    transpose_round(ch)

    nc.compile()
    return nc


def _prep_inputs(x, qweight, lut, rows, cols, vals, bias):
    x = np.asarray(x, dtype=np.float32)
    qweight = np.asarray(qweight, dtype=np.int32)
    lut = np.asarray(lut, dtype=np.float32)
    rows = np.asarray(rows, dtype=np.int64)
    cols = np.asarray(cols, dtype=np.int64)
    vals = np.asarray(vals, dtype=np.float32)
    bias = np.asarray(bias, dtype=np.float32)

    idx = _host_indices(qweight)
    tbl, cst, cva, CSE, NI, SC16 = _scatter_tables(idx, rows, cols, vals)

    # x -> [chunk, i, token] bf16 (pure relayout + dtype rounding)
    xt = np.ascontiguousarray(
        x.reshape(NCHUNK, 128, IC, 128).transpose(0, 3, 2, 1)
    ).reshape(NCHUNK, 128, I).astype(ml_dtypes.bfloat16)

    in_maps = []
    for c in range(N_CORES):
        osl = slice(O_SH * c, O_SH * (c + 1))
        in_maps.append(
            {
                "x": xt,
                "lut": np.ascontiguousarray(lut[osl].reshape(OT, 128, 16)),
                "qidx": np.ascontiguousarray(
                    tbl[osl].reshape(OT, 128, -1)
                ),
                "cst": np.ascontiguousarray(cst[osl].reshape(OT, 128, SC16)),
                "cvals": np.ascontiguousarray(cva[osl].reshape(OT, 128, -1)),
                "bias": np.ascontiguousarray(bias[osl].reshape(1, O_SH)),
                "eye": _EYE,
            }
        )
    return in_maps, CSE, NI, SC16


def _run(inputs, trace=False, trace_kwargs=None):
    from concourse.bass_utils import run_bass_kernel_spmd

    in_maps, CSE, NI, SC16 = _prep_inputs(**inputs)

    key = (CSE, NI, SC16)
    if key not in _GRAPH_CACHE:
        _GRAPH_CACHE[key] = _build_graph(CSE, NI, SC16)
    nc = _GRAPH_CACHE[key]

    res = run_bass_kernel_spmd(
        nc, in_maps, core_ids=list(range(N_CORES)),
        trace=trace, **(trace_kwargs or {}),
    )
    out = np.empty((NT, O), np.float32)
    for c in range(N_CORES):
        yc = res.results[c]["y"].reshape(NT, O_SH)
        out[:, O_SH * c : O_SH * (c + 1)] = yc
    return out.reshape(B, S, O), res


def kernel(x, qweight, lut, rows, cols, vals, bias):
    out, _ = _run(dict(x=x, qweight=qweight, lut=lut, rows=rows,
                       cols=cols, vals=vals, bias=bias))
    return out


# revision 29
# speedup vs baseline: 2.8534x; 1.0090x over previous
"""AnyPrecisionLinear (4-bit LUT dequant + CSR outliers + bias) on 8 TRN2 cores.

Sharding: 8-way over out_features (O); tokens replicated.
Core c handles o in [512*c, 512*(c+1)), all 8192 tokens.

Device does all value math:
  - W rows built from lut via GPSIMD local_scatter of full LUT values
    (host precomputes pure index slot tables from qweight bits).
  - CSR outlier values: lut part selected by one tiny local_scatter from the
    replicated-lut pattern, added to DMA'd CSR values on DVE, merged into the
    same per-chunk scatter.
  - W transposed on the PE (is_transpose matmul), 4 blocks per PSUM tile.
  - GEMM on TensorE (bf16, f32 PSUM accum); bias added on DVE at copy-out
    from a partition_broadcast bias row.
Host does only layout/index work: sharding, bit-plane->index repack, slot
tables, CSR indptr parsing + dedup, x transpose to [chunk, i, token] layout
(+ f32->bf16 rounding), output concat.
"""

import numpy as np
from contextlib import ExitStack

import ml_dtypes

# Problem constants (hardcoded per harness contract).
B, S, I, O = 4, 2048, 4096, 4096
W_BITS = 4
NT = B * S                # 8192 tokens
N_CORES = 8
O_SH = O // N_CORES       # 512 out features per core
OT = O_SH // 128          # 4 o-tiles of 128 rows per core
NCHUNK = NT // 128        # 64 token chunks
IC = I // 128             # 32 i-blocks
CH = 1024                 # i-chunk size for local_scatter
NCH = I // CH             # 4

XB = 2                    # x chunks per DMA
YB = 4                    # y chunks per DMA store

_GRAPH_CACHE = {}

_EYE = np.eye(128, dtype=ml_dtypes.bfloat16)


def _host_indices(qweight):
    """bit-planes -> 4-bit index array [O, I] (uint8). Pure bit relayout."""
    shifts = np.arange(32, dtype=np.int32)
    bits = ((qweight[:, :, :, None] >> shifts) & 1).astype(np.uint8)
    planew = (1 << (W_BITS - 1 - np.arange(W_BITS))).astype(np.uint8)
    idx = (bits * planew[:, None, None, None]).sum(axis=0, dtype=np.int32)
    return idx.reshape(O, I).astype(np.uint8)


def _scatter_tables(idx, rows, cols, vals):
    """Slot tables for the merged dequant+CSR local_scatter.

    Per o-row the device holds one data strip [4*CSE + NI]:
      [0 : 4*CSE)        comb slots: chunk-major CSR values (cv + lut[v])
      [4*CSE : 4*CSE+NI) pattern slots: slot 16*s+v holds lut[o, v]
    Chunk ch's scatter uses idx table tbl[o, ch] over the whole strip; slots
    belonging to other chunks (or unused) are -1.

    Returns:
      tbl   [O, NCH, W] int16  scatter dest (position in chunk) or -1
      cst   [O, SC16]   int16  tiny-scatter dest (comb slot) for CSR lut part
      cva   [O, NCH*CSE] f32   CSR values (0 pad)
      CSE, NI, SC16
    """
    nnz = cols.shape[0]
    row_ids = (np.searchsorted(rows, np.arange(nnz), side="right") - 1).astype(np.int64)
    key = row_ids * I + cols.astype(np.int64)
    uk, inv = np.unique(key, return_inverse=True)
    v2 = np.zeros(len(uk), np.float64)
    np.add.at(v2, inv, vals.astype(np.float64))
    r2 = uk // I
    c2 = uk % I
    ch2 = (c2 // CH).astype(np.int64)
    cl2 = (c2 % CH).astype(np.int16)

    grp = r2 * NCH + ch2                       # ascending (uk sorted)
    _, gstart, gcount = np.unique(grp, return_index=True, return_counts=True)
    CSE = int(gcount.max())
    CSE += CSE % 2
    CSE = max(CSE, 2)
    rank = np.arange(len(uk)) - np.repeat(gstart, gcount)

    is_csr = np.zeros((O, NCH, CH), bool)
    is_csr[r2, ch2, cl2] = True

    # ---- dequant slots: all 16 values, csr positions excluded ----
    idx4 = idx.reshape(O, NCH, CH).astype(np.int16)
    idxm = np.where(is_csr, np.int16(16), idx4)          # sentinel sorts last
    order = np.argsort(idxm, axis=-1, kind="stable").astype(np.int16)
    sortedv = np.take_along_axis(idxm, order.astype(np.int64), axis=-1)
    keep = sortedv < 16
    cnt = np.zeros((O, NCH, 16), np.int32)
    for v in range(16):
        cnt[:, :, v] = (idxm == v).sum(-1)
    Smax = int(cnt.max())
    NI = 16 * Smax
    cstart = np.concatenate(
        [np.zeros((O, NCH, 1), np.int32), np.cumsum(cnt, -1)[:, :, :-1]], -1
    )
    srank = np.arange(CH)[None, None, :] - np.take_along_axis(
        cstart, np.minimum(sortedv, 15).astype(np.int64), axis=-1
    )
    W = 4 * CSE + NI
    tbl = np.full((O, NCH, W + 2), -1, np.int16)
    slot = (4 * CSE + 16 * srank + sortedv).astype(np.int64)
    np.put_along_axis(
        tbl, np.where(keep, slot, W + 1),
        np.where(keep, order, -1), axis=-1,
    )
    tbl = tbl[:, :, :W].copy()

    # ---- csr dest slots in the per-chunk tables ----
    comb_slot = (ch2 * CSE + rank).astype(np.int64)
    tbl[r2, ch2, comb_slot] = cl2

    # ---- csr values + tiny-scatter table (lut part of comb) ----
    cva = np.zeros((O, NCH * CSE), np.float32)
    cva[r2, comb_slot] = v2.astype(np.float32)
    vsl = idx4[r2, ch2, cl2.astype(np.int64)]            # lut index per entry
    # occurrence rank of (row, v) among csr entries of that row
    keyrv = r2 * 16 + vsl
    ord2 = np.argsort(keyrv, kind="stable")
    kr_sorted = keyrv[ord2]
    _, g2start, g2count = np.unique(kr_sorted, return_index=True, return_counts=True)
    rank2 = np.empty(len(uk), np.int64)
    rank2[ord2] = np.arange(len(uk)) - np.repeat(g2start, g2count)
    SC = max(int(g2count.max()), 1)
    SC16 = 16 * SC
    cst = np.full((O, SC16), -1, np.int16)
    cst[r2, 16 * rank2 + vsl] = comb_slot.astype(np.int16)
    return tbl, cst, cva, CSE, NI, SC16


def _build_graph(CSE, NI, SC16):
    import concourse.bass as bass
    import concourse.bacc as bacc
    import concourse.tile as tile
    from concourse import mybir

    dt = mybir.dt
    nc = bacc.Bacc("TRN2", target_bir_lowering=False, debug=False)

    WCOL = 4 * CSE + NI
    x_d = nc.dram_tensor("x", [NCHUNK, 128, I], dt.bfloat16, kind="ExternalInput")
    lut_d = nc.dram_tensor("lut", [OT, 128, 16], dt.float32, kind="ExternalInput")
    qid_d = nc.dram_tensor("qidx", [OT, 128, NCH * WCOL], dt.int16, kind="ExternalInput")
    cst_d = nc.dram_tensor("cst", [OT, 128, SC16], dt.int16, kind="ExternalInput")
    cva_d = nc.dram_tensor("cvals", [OT, 128, NCH * CSE], dt.float32, kind="ExternalInput")
    bias_d = nc.dram_tensor("bias", [1, O_SH], dt.float32, kind="ExternalInput")
    eye_d = nc.dram_tensor("eye", [128, 128], dt.bfloat16, kind="ExternalInput")
    y_d = nc.dram_tensor("y", [NCHUNK, 128, O_SH], dt.float32, kind="ExternalOutput")

    # GEMM phases over i-block ranges; phase k covers scatter-chunk chs PH[k].
    # Matmuls of chunk PAIRS interleave on two PSUM banks to hide the PE
    # array drain (serial fill->drain on one bank costs ~46ns/matmul).
    PH = [(0, 8), (8, 32)]             # [ic0, ic1) per phase
    # transpose-round insertion points: (phase, pair-index) -> ch
    TINS = {(0, 10): 1, (0, 19): 2, (0, 28): 3}

    with tile.TileContext(nc) as tc, ExitStack() as ctx:
        const = ctx.enter_context(tc.tile_pool(name="const", bufs=1))
        dpool = ctx.enter_context(tc.tile_pool(name="dp", bufs=1))
        qpool = ctx.enter_context(tc.tile_pool(name="qp", bufs=6))
        spool = ctx.enter_context(tc.tile_pool(name="sp", bufs=2))
        wpool = ctx.enter_context(tc.tile_pool(name="w", bufs=12))
        xpool = ctx.enter_context(tc.tile_pool(name="x", bufs=3))
        ypool = ctx.enter_context(tc.tile_pool(name="ya", bufs=2))
        y2pool = ctx.enter_context(tc.tile_pool(name="yb", bufs=3))
        psum = ctx.enter_context(
            tc.tile_pool(name="ps", bufs=4, space=bass.MemorySpace.PSUM)
        )
        pst = ctx.enter_context(
            tc.tile_pool(name="pst", bufs=2, space=bass.MemorySpace.PSUM)
        )

        # Resident transposed weights: WT[p, 512*ic + 128*t + ol] = W[128*t+ol, 128*ic+p]
        WT = const.tile([128, IC * O_SH], dt.bfloat16)

        eye = const.tile([128, 128], dt.bfloat16)
        nc.scalar.dma_start(eye[:, :], eye_d[:, :])

        # ---- per-tile preps: pattern + CSR comb values ----
        datas, wqs = [], []

        def dequant_one(ch, t, dma_eng):
            qix = qpool.tile([128, WCOL], dt.int16, tag="qix")
            dma_eng.dma_start(
                qix[:, :], qid_d[t][:, ch * WCOL : (ch + 1) * WCOL]
            )
            wq = wpool.tile([128, CH], dt.bfloat16, tag="wq")
            nc.gpsimd.local_scatter(
                wq[:, :], datas[t][:, :], qix[:, :],
                channels=128, num_elems=CH, num_idxs=WCOL,
            )
            wqs.append(wq)

        # ch0 scatter tables load first so the first scatter isn't queued
        # behind the prep DMAs.
        qix0 = []
        for t in range(OT):
            qix = qpool.tile([128, WCOL], dt.int16, tag="qix")
            nc.sync.dma_start(qix[:, :], qid_d[t][:, 0:WCOL])
            qix0.append(qix)

        for t in range(OT):
            lutf = spool.tile([128, 16], dt.float32, tag="lutf")
            nc.sync.dma_start(lutf[:, :], lut_d[t])
            data = dpool.tile([128, WCOL], dt.bfloat16, tag=f"data{t}")
            nc.vector.tensor_copy(data[:, 4 * CSE : 4 * CSE + 16], lutf[:, :])
            sz = 16
            while sz < NI:
                cp = min(sz, NI - sz)
                nc.vector.tensor_copy(
                    data[:, 4 * CSE + sz : 4 * CSE + sz + cp],
                    data[:, 4 * CSE : 4 * CSE + cp],
                )
                sz += cp
            cstt = spool.tile([128, SC16], dt.int16, tag="cst")
            nc.sync.dma_start(cstt[:, :], cst_d[t])
            nc.gpsimd.local_scatter(
                data[:, 0 : 4 * CSE], data[:, 4 * CSE : 4 * CSE + SC16],
                cstt[:, :], channels=128, num_elems=4 * CSE, num_idxs=SC16,
            )
            cvf = spool.tile([128, NCH * CSE], dt.float32, tag="cvf")
            nc.sync.dma_start(cvf[:, :], cva_d[t])
            cvb = spool.tile([128, NCH * CSE], dt.bfloat16, tag="cvb")
            nc.vector.tensor_copy(cvb[:, :], cvf[:, :])
            nc.vector.tensor_add(
                data[:, 0 : 4 * CSE], data[:, 0 : 4 * CSE], cvb[:, :]
            )
            datas.append(data)

        browp = const.tile([1, O_SH], dt.float32)
        nc.scalar.dma_start(browp[:, :], bias_d[:, :])
        brow = const.tile([128, O_SH], dt.float32)
        nc.gpsimd.partition_broadcast(brow[:, :], browp[:, :])

        for t in range(OT):
            wq = wpool.tile([128, CH], dt.bfloat16, tag="wq")
            nc.gpsimd.local_scatter(
                wq[:, :], datas[t][:, :], qix0[t][:, :],
                channels=128, num_elems=CH, num_idxs=WCOL,
            )
            wqs.append(wq)

        def transpose_round(ch):
            for t in range(OT):
                wq = wqs[4 * ch + t]
                for g in range(2):
                    pt = pst.tile([128, 512], dt.bfloat16, tag="pt")
                    for k in range(4):
                        l = 4 * g + k
                        nc.tensor.transpose(
                            pt[:, 128 * k : 128 * (k + 1)],
                            wq[:, 128 * l : 128 * (l + 1)],
                            eye[:, :],
                        )
                    ic0 = 8 * ch + 4 * g
                    dst = WT[:, :].rearrange("p (ic o) -> p ic o", o=O_SH)[
                        :, ic0 : ic0 + 4, 128 * t : 128 * (t + 1)
                    ]
                    nc.vector.tensor_copy(
                        dst, pt[:, :].rearrange("p (a b) -> p a b", b=128)
                    )

        # ch1-3 table loads stream on the ACT queue while the GEMM runs.
        for ch in range(1, NCH):
            for t in range(OT):
                dequant_one(ch, t, nc.scalar)
        transpose_round(0)

        # ---- GEMM: 3 phases over i-blocks; chunk pairs share the PE ----
        for ph, (ica, icb) in enumerate(PH):
            c0, c1 = 128 * ica, 128 * icb
            first, last = ph == 0, ph == len(PH) - 1
            for p in range(NCHUNK // 2):
                n0 = 2 * p
                xT = xpool.tile([128, 2, c1 - c0], dt.bfloat16, tag="xT")
                nc.sync.dma_start(
                    xT[:, :, :],
                    x_d[n0 : n0 + 2][:, :, c0:c1].rearrange("a b c -> b a c"),
                )
                if n0 % YB == 0:
                    pool = ypool if first else y2pool
                    yo = pool.tile([128, YB, O_SH], dt.float32, tag="yo")
                ps0 = psum.tile([128, O_SH], dt.float32, tag="ps")
                ps1 = psum.tile([128, O_SH], dt.float32, tag="ps")
                pss = [ps0, ps1]
                for ic in range(ica, icb):
                    for j in range(2):
                        nc.tensor.matmul(
                            pss[j][:, :],
                            xT[:, j, 128 * ic - c0 : 128 * (ic + 1) - c0],
                            WT[:, O_SH * ic : O_SH * (ic + 1)],
                            start=(ic == ica), stop=(ic == icb - 1),
                        )
                for j in range(2):
                    if first:
                        nc.vector.tensor_add(
                            yo[:, (n0 + j) % YB, :], pss[j][:, :], brow[:, :]
                        )
                    else:
                        nc.vector.tensor_copy(yo[:, (n0 + j) % YB, :], pss[j][:, :])
                if last and n0 + 4 >= NCHUNK:
                    # split the final batch into 2-chunk stores to trim the tail
                    half = 0 if n0 + 4 == NCHUNK else 1
                    nc.gpsimd.dma_start(
                        y_d[n0 : n0 + 2].rearrange("a b c -> b a c"),
                        yo[:, 2 * half : 2 * half + 2, :],
                        accum_op=mybir.AluOpType.add,
                    )
                elif (n0 + 1) % YB == YB - 1:
                    ysl = y_d[n0 + 2 - YB : n0 + 2].rearrange("a b c -> b a c")
                    if first:
                        nc.scalar.dma_start(ysl, yo[:, :, :])
                    else:
                        nc.gpsimd.dma_start(
                            ysl, yo[:, :, :], accum_op=mybir.AluOpType.add
                        )
                ch = TINS.get((ph, p))
                if ch is not None:
                    transpose_round(ch)

    nc.compile()
    return nc


def _prep_inputs(x, qweight, lut, rows, cols, vals, bias):
    x = np.asarray(x, dtype=np.float32)
    qweight = np.asarray(qweight, dtype=np.int32)
    lut = np.asarray(lut, dtype=np.float32)
    rows = np.asarray(rows, dtype=np.int64)
    cols = np.asarray(cols, dtype=np.int64)
    vals = np.asarray(vals, dtype=np.float32)
    bias = np.asarray(bias, dtype=np.float32)

    idx = _host_indices(qweight)
    tbl, cst, cva, CSE, NI, SC16 = _scatter_tables(idx, rows, cols, vals)

    # x -> [chunk, i, token] bf16 (pure relayout + dtype rounding)
    xt = np.ascontiguousarray(
        x.reshape(NCHUNK, 128, IC, 128).transpose(0, 3, 2, 1)
    ).reshape(NCHUNK, 128, I).astype(ml_dtypes.bfloat16)

    in_maps = []
    for c in range(N_CORES):
        osl = slice(O_SH * c, O_SH * (c + 1))
        in_maps.append(
            {
                "x": xt,
                "lut": np.ascontiguousarray(lut[osl].reshape(OT, 128, 16)),
                "qidx": np.ascontiguousarray(
                    tbl[osl].reshape(OT, 128, -1)
                ),
                "cst": np.ascontiguousarray(cst[osl].reshape(OT, 128, SC16)),
                "cvals": np.ascontiguousarray(cva[osl].reshape(OT, 128, -1)),
                "bias": np.ascontiguousarray(bias[osl].reshape(1, O_SH)),
                "eye": _EYE,
            }
        )
    return in_maps, CSE, NI, SC16


def _run(inputs, trace=False, trace_kwargs=None):
    from concourse.bass_utils import run_bass_kernel_spmd

    in_maps, CSE, NI, SC16 = _prep_inputs(**inputs)

    key = (CSE, NI, SC16)
    if key not in _GRAPH_CACHE:
        _GRAPH_CACHE[key] = _build_graph(CSE, NI, SC16)
    nc = _GRAPH_CACHE[key]

    res = run_bass_kernel_spmd(
        nc, in_maps, core_ids=list(range(N_CORES)),
        trace=trace, **(trace_kwargs or {}),
    )
    out = np.empty((NT, O), np.float32)
    for c in range(N_CORES):
        yc = res.results[c]["y"].reshape(NT, O_SH)
        out[:, O_SH * c : O_SH * (c + 1)] = yc
    return out.reshape(B, S, O), res


def kernel(x, qweight, lut, rows, cols, vals, bias):
    out, _ = _run(dict(x=x, qweight=qweight, lut=lut, rows=rows,
                       cols=cols, vals=vals, bias=bias))
    return out


# revision 36
# speedup vs baseline: 3.0401x; 1.0654x over previous
"""AnyPrecisionLinear (4-bit LUT dequant + CSR outliers + bias) on 8 TRN2 cores.

Sharding: 8-way over out_features (O); tokens replicated.
Core c handles o in [512*c, 512*(c+1)), all 8192 tokens.

Device does all value math:
  - W rows built from lut via GPSIMD local_scatter of full LUT values
    (host precomputes pure index slot tables from qweight bits).
  - CSR outlier values: lut part selected by one tiny local_scatter from the
    replicated-lut pattern, added to DMA'd CSR values on DVE, merged into the
    same per-chunk scatter.
  - W transposed on the PE (is_transpose matmul), 4 blocks per PSUM tile.
  - GEMM on TensorE (bf16, f32 PSUM accum); bias added on DVE at copy-out
    from a partition_broadcast bias row.
Host does only layout/index work: sharding, bit-plane->index repack, slot
tables, CSR indptr parsing + dedup, x transpose to [chunk, i, token] layout
(+ f32->bf16 rounding), output concat.
"""

import numpy as np
from contextlib import ExitStack

import ml_dtypes

# Problem constants (hardcoded per harness contract).
B, S, I, O = 4, 2048, 4096, 4096
W_BITS = 4
NT = B * S                # 8192 tokens
N_CORES = 8
O_SH = O // N_CORES       # 512 out features per core
OT = O_SH // 128          # 4 o-tiles of 128 rows per core
NCHUNK = NT // 128        # 64 token chunks
IC = I // 128             # 32 i-blocks
CH = 1024                 # i-chunk size for local_scatter
NCH = I // CH             # 4

XB = 2                    # x chunks per DMA
YB = 4                    # y chunks per DMA store

_GRAPH_CACHE = {}

_EYE = np.eye(128, dtype=ml_dtypes.bfloat16)


def _host_indices(qweight):
    """bit-planes -> 4-bit index array [O, I] (uint8). Pure bit relayout."""
    shifts = np.arange(32, dtype=np.int32)
    bits = ((qweight[:, :, :, None] >> shifts) & 1).astype(np.uint8)
    planew = (1 << (W_BITS - 1 - np.arange(W_BITS))).astype(np.uint8)
    idx = (bits * planew[:, None, None, None]).sum(axis=0, dtype=np.int32)
    return idx.reshape(O, I).astype(np.uint8)


def _scatter_tables(idx, rows, cols, vals):
    """Slot tables for the merged dequant+CSR local_scatter.

    Per o-row the device holds one data strip [4*CSE + NI]:
      [0 : 4*CSE)        comb slots: chunk-major CSR values (cv + lut[v])
      [4*CSE : 4*CSE+NI) pattern slots: slot 16*s+v holds lut[o, v]
    Chunk ch's scatter uses idx table tbl[o, ch] over the whole strip; slots
    belonging to other chunks (or unused) are -1.

    Returns:
      tbl   [O, NCH, W] int16  scatter dest (position in chunk) or -1
      cst   [O, SC16]   int16  tiny-scatter dest (comb slot) for CSR lut part
      cva   [O, NCH*CSE] f32   CSR values (0 pad)
      CSE, NI, SC16
    """
    nnz = cols.shape[0]
    row_ids = (np.searchsorted(rows, np.arange(nnz), side="right") - 1).astype(np.int64)
    key = row_ids * I + cols.astype(np.int64)
    uk, inv = np.unique(key, return_inverse=True)
    v2 = np.zeros(len(uk), np.float64)
    np.add.at(v2, inv, vals.astype(np.float64))
    r2 = uk // I
    c2 = uk % I
    ch2 = (c2 // CH).astype(np.int64)
    cl2 = (c2 % CH).astype(np.int16)

    grp = r2 * NCH + ch2                       # ascending (uk sorted)
    _, gstart, gcount = np.unique(grp, return_index=True, return_counts=True)
    CSE = int(gcount.max())
    CSE += CSE % 2
    CSE = max(CSE, 2)
    rank = np.arange(len(uk)) - np.repeat(gstart, gcount)

    is_csr = np.zeros((O, NCH, CH), bool)
    is_csr[r2, ch2, cl2] = True

    # ---- dequant slots: all 16 values, csr positions excluded ----
    idx4 = idx.reshape(O, NCH, CH).astype(np.int16)
    idxm = np.where(is_csr, np.int16(16), idx4)          # sentinel sorts last
    order = np.argsort(idxm, axis=-1, kind="stable").astype(np.int16)
    sortedv = np.take_along_axis(idxm, order.astype(np.int64), axis=-1)
    keep = sortedv < 16
    cnt = np.zeros((O, NCH, 16), np.int32)
    for v in range(16):
        cnt[:, :, v] = (idxm == v).sum(-1)
    Smax = int(cnt.max())
    NI = 16 * Smax
    cstart = np.concatenate(
        [np.zeros((O, NCH, 1), np.int32), np.cumsum(cnt, -1)[:, :, :-1]], -1
    )
    srank = np.arange(CH)[None, None, :] - np.take_along_axis(
        cstart, np.minimum(sortedv, 15).astype(np.int64), axis=-1
    )
    W = 4 * CSE + NI
    tbl = np.full((O, NCH, W + 2), -1, np.int16)
    slot = (4 * CSE + 16 * srank + sortedv).astype(np.int64)
    np.put_along_axis(
        tbl, np.where(keep, slot, W + 1),
        np.where(keep, order, -1), axis=-1,
    )
    tbl = tbl[:, :, :W].copy()

    # ---- csr dest slots in the per-chunk tables ----
    comb_slot = (ch2 * CSE + rank).astype(np.int64)
    tbl[r2, ch2, comb_slot] = cl2

    # ---- csr values + tiny-scatter table (lut part of comb) ----
    cva = np.zeros((O, NCH * CSE), np.float32)
    cva[r2, comb_slot] = v2.astype(np.float32)
    vsl = idx4[r2, ch2, cl2.astype(np.int64)]            # lut index per entry
    # occurrence rank of (row, v) among csr entries of that row
    keyrv = r2 * 16 + vsl
    ord2 = np.argsort(keyrv, kind="stable")
    kr_sorted = keyrv[ord2]
    _, g2start, g2count = np.unique(kr_sorted, return_index=True, return_counts=True)
    rank2 = np.empty(len(uk), np.int64)
    rank2[ord2] = np.arange(len(uk)) - np.repeat(g2start, g2count)
    SC = max(int(g2count.max()), 1)
    SC16 = 16 * SC
    cst = np.full((O, SC16), -1, np.int16)
    cst[r2, 16 * rank2 + vsl] = comb_slot.astype(np.int16)
    return tbl, cst, cva, CSE, NI, SC16


def _build_graph(CSE, NI, SC16):
    import concourse.bass as bass
    import concourse.bacc as bacc
    import concourse.tile as tile
    from concourse import mybir

    dt = mybir.dt
    nc = bacc.Bacc("TRN2", target_bir_lowering=False, debug=False)

    WCOL = 4 * CSE + NI
    x_d = nc.dram_tensor("x", [NCHUNK, 128, I], dt.bfloat16, kind="ExternalInput")
    lut_d = nc.dram_tensor("lut", [OT, 128, 16], dt.float32, kind="ExternalInput")
    qid_d = nc.dram_tensor("qidx", [NCH, OT, 128, WCOL], dt.int16, kind="ExternalInput")
    cst_d = nc.dram_tensor("cst", [OT, 128, SC16], dt.int16, kind="ExternalInput")
    cva_d = nc.dram_tensor("cvals", [OT, 128, NCH * CSE], dt.float32, kind="ExternalInput")
    bias_d = nc.dram_tensor("bias", [1, O_SH], dt.float32, kind="ExternalInput")
    eye_d = nc.dram_tensor("eye", [128, 128], dt.bfloat16, kind="ExternalInput")
    y_d = nc.dram_tensor("y", [NCHUNK, 128, O_SH], dt.float32, kind="ExternalOutput")

    # GEMM phases over i-block ranges; phase k covers scatter-chunk chs PH[k].
    # Matmuls of chunk PAIRS interleave on two PSUM banks to hide the PE
    # array drain (serial fill->drain on one bank costs ~46ns/matmul).
    PH = [(0, 8), (8, 32)]             # [ic0, ic1) per phase
    # transpose-round insertion points: (phase, pair-index) -> ch
    TINS = {(0, 10): 1, (0, 19): 2, (0, 28): 3}

    with tile.TileContext(nc) as tc, ExitStack() as ctx:
        const = ctx.enter_context(tc.tile_pool(name="const", bufs=1))
        dpool = ctx.enter_context(tc.tile_pool(name="dp", bufs=1))
        qpool = ctx.enter_context(tc.tile_pool(name="qp", bufs=3))
        spool = ctx.enter_context(tc.tile_pool(name="sp", bufs=2))
        wpool = ctx.enter_context(tc.tile_pool(name="w", bufs=12))
        xpool = ctx.enter_context(tc.tile_pool(name="x", bufs=3))
        ypool = ctx.enter_context(tc.tile_pool(name="ya", bufs=2))
        y2pool = ctx.enter_context(tc.tile_pool(name="yb", bufs=2))
        psum = ctx.enter_context(
            tc.tile_pool(name="ps", bufs=4, space=bass.MemorySpace.PSUM)
        )
        pst = ctx.enter_context(
            tc.tile_pool(name="pst", bufs=2, space=bass.MemorySpace.PSUM)
        )

        # Resident transposed weights: WT[p, 512*ic + 128*t + ol] = W[128*t+ol, 128*ic+p]
        WT = const.tile([128, IC * O_SH], dt.bfloat16)

        eye = const.tile([128, 128], dt.bfloat16)
        nc.scalar.dma_start(eye[:, :], eye_d[:, :])

        # ---- per-tile preps: pattern + CSR comb values ----
        datas, wqs = [], []

        def qround_load(ch, dma_eng):
            qr = qpool.tile([128, OT, WCOL], dt.int16, tag="qr")
            dma_eng.dma_start(qr[:, :, :], qid_d[ch].rearrange("t p w -> p t w"))
            return qr

        def scatter_round(ch, qr):
            for t in range(OT):
                wq = wpool.tile([128, CH], dt.bfloat16, tag="wq")
                nc.gpsimd.local_scatter(
                    wq[:, :], datas[t][:, :], qr[:, t, :],
                    channels=128, num_elems=CH, num_idxs=WCOL,
                )
                wqs.append(wq)

        # ch0 scatter tables load first so the first scatter isn't queued
        # behind the prep DMAs.
        qr0 = qround_load(0, nc.sync)

        for t in range(OT):
            lutf = spool.tile([128, 16], dt.float32, tag="lutf")
            nc.sync.dma_start(lutf[:, :], lut_d[t])
            data = dpool.tile([128, WCOL], dt.bfloat16, tag=f"data{t}")
            nc.vector.tensor_copy(data[:, 4 * CSE : 4 * CSE + 16], lutf[:, :])
            sz = 16
            while sz < NI:
                cp = min(sz, NI - sz)
                nc.vector.tensor_copy(
                    data[:, 4 * CSE + sz : 4 * CSE + sz + cp],
                    data[:, 4 * CSE : 4 * CSE + cp],
                )
                sz += cp
            cstt = spool.tile([128, SC16], dt.int16, tag="cst")
            nc.sync.dma_start(cstt[:, :], cst_d[t])
            nc.gpsimd.local_scatter(
                data[:, 0 : 4 * CSE], data[:, 4 * CSE : 4 * CSE + SC16],
                cstt[:, :], channels=128, num_elems=4 * CSE, num_idxs=SC16,
            )
            cvf = spool.tile([128, NCH * CSE], dt.float32, tag="cvf")
            nc.sync.dma_start(cvf[:, :], cva_d[t])
            cvb = spool.tile([128, NCH * CSE], dt.bfloat16, tag="cvb")
            nc.vector.tensor_copy(cvb[:, :], cvf[:, :])
            nc.vector.tensor_add(
                data[:, 0 : 4 * CSE], data[:, 0 : 4 * CSE], cvb[:, :]
            )
            datas.append(data)

        browp = const.tile([1, O_SH], dt.float32)
        nc.scalar.dma_start(browp[:, :], bias_d[:, :])
        brow = const.tile([128, O_SH], dt.float32)
        nc.gpsimd.partition_broadcast(brow[:, :], browp[:, :])

        scatter_round(0, qr0)

        def transpose_round(ch):
            for t in range(OT):
                wq = wqs[4 * ch + t]
                for g in range(2):
                    pt = pst.tile([128, 512], dt.bfloat16, tag="pt")
                    for k in range(4):
                        l = 4 * g + k
                        nc.tensor.transpose(
                            pt[:, 128 * k : 128 * (k + 1)],
                            wq[:, 128 * l : 128 * (l + 1)],
                            eye[:, :],
                        )
                    ic0 = 8 * ch + 4 * g
                    dst = WT[:, :].rearrange("p (ic o) -> p ic o", o=O_SH)[
                        :, ic0 : ic0 + 4, 128 * t : 128 * (t + 1)
                    ]
                    nc.vector.tensor_copy(
                        dst, pt[:, :].rearrange("p (a b) -> p a b", b=128)
                    )

        # ch1-3 table round-loads go on the ACT queue (3 quick DMAs, done
        # before the y stores start).
        for ch in range(1, NCH):
            qr = qround_load(ch, nc.scalar)
            scatter_round(ch, qr)
        transpose_round(0)

        # ---- GEMM: 3 phases over i-blocks; chunk pairs share the PE ----
        for ph, (ica, icb) in enumerate(PH):
            c0, c1 = 128 * ica, 128 * icb
            first, last = ph == 0, ph == len(PH) - 1
            for p in range(NCHUNK // 2):
                n0 = 2 * p
                xT = xpool.tile([128, 2, c1 - c0], dt.bfloat16, tag="xT")
                nc.sync.dma_start(
                    xT[:, :, :],
                    x_d[n0 : n0 + 2][:, :, c0:c1].rearrange("a b c -> b a c"),
                )
                if n0 % YB == 0:
                    pool = ypool if first else y2pool
                    yo = pool.tile([128, YB, O_SH], dt.float32, tag="yo")
                ps0 = psum.tile([128, O_SH], dt.float32, tag="ps")
                ps1 = psum.tile([128, O_SH], dt.float32, tag="ps")
                pss = [ps0, ps1]
                for ic in range(ica, icb):
                    for j in range(2):
                        nc.tensor.matmul(
                            pss[j][:, :],
                            xT[:, j, 128 * ic - c0 : 128 * (ic + 1) - c0],
                            WT[:, O_SH * ic : O_SH * (ic + 1)],
                            start=(ic == ica), stop=(ic == icb - 1),
                        )
                for j in range(2):
                    if first:
                        nc.vector.tensor_add(
                            yo[:, (n0 + j) % YB, :], pss[j][:, :], brow[:, :]
                        )
                    else:
                        nc.vector.tensor_copy(yo[:, (n0 + j) % YB, :], pss[j][:, :])
                if last and n0 + 4 >= NCHUNK:
                    # split the final batch into 2-chunk stores to trim the tail
                    half = 0 if n0 + 4 == NCHUNK else 1
                    nc.gpsimd.dma_start(
                        y_d[n0 : n0 + 2].rearrange("a b c -> b a c"),
                        yo[:, 2 * half : 2 * half + 2, :],
                        accum_op=mybir.AluOpType.add,
                    )
                elif (n0 + 1) % YB == YB - 1:
                    ysl = y_d[n0 + 2 - YB : n0 + 2].rearrange("a b c -> b a c")
                    if first:
                        nc.scalar.dma_start(ysl, yo[:, :, :])
                    else:
                        nc.gpsimd.dma_start(
                            ysl, yo[:, :, :], accum_op=mybir.AluOpType.add
                        )
                ch = TINS.get((ph, p))
                if ch is not None:
                    transpose_round(ch)

    nc.compile()
    return nc


def _prep_inputs(x, qweight, lut, rows, cols, vals, bias):
    x = np.asarray(x, dtype=np.float32)
    qweight = np.asarray(qweight, dtype=np.int32)
    lut = np.asarray(lut, dtype=np.float32)
    rows = np.asarray(rows, dtype=np.int64)
    cols = np.asarray(cols, dtype=np.int64)
    vals = np.asarray(vals, dtype=np.float32)
    bias = np.asarray(bias, dtype=np.float32)

    idx = _host_indices(qweight)
    tbl, cst, cva, CSE, NI, SC16 = _scatter_tables(idx, rows, cols, vals)

    # x -> [chunk, i, token] bf16 (pure relayout + dtype rounding)
    xt = np.ascontiguousarray(
        x.reshape(NCHUNK, 128, IC, 128).transpose(0, 3, 2, 1)
    ).reshape(NCHUNK, 128, I).astype(ml_dtypes.bfloat16)

    in_maps = []
    for c in range(N_CORES):
        osl = slice(O_SH * c, O_SH * (c + 1))
        in_maps.append(
            {
                "x": xt,
                "lut": np.ascontiguousarray(lut[osl].reshape(OT, 128, 16)),
                "qidx": np.ascontiguousarray(
                    tbl[osl].reshape(OT, 128, NCH, -1).transpose(2, 0, 1, 3)
                ),
                "cst": np.ascontiguousarray(cst[osl].reshape(OT, 128, SC16)),
                "cvals": np.ascontiguousarray(cva[osl].reshape(OT, 128, -1)),
                "bias": np.ascontiguousarray(bias[osl].reshape(1, O_SH)),
                "eye": _EYE,
            }
        )
    return in_maps, CSE, NI, SC16


def _run(inputs, trace=False, trace_kwargs=None):
    from concourse.bass_utils import run_bass_kernel_spmd

    in_maps, CSE, NI, SC16 = _prep_inputs(**inputs)

    key = (CSE, NI, SC16)
    if key not in _GRAPH_CACHE:
        _GRAPH_CACHE[key] = _build_graph(CSE, NI, SC16)
    nc = _GRAPH_CACHE[key]

    res = run_bass_kernel_spmd(
        nc, in_maps, core_ids=list(range(N_CORES)),
        trace=trace, **(trace_kwargs or {}),
    )
    out = np.empty((NT, O), np.float32)
    for c in range(N_CORES):
        yc = res.results[c]["y"].reshape(NT, O_SH)
        out[:, O_SH * c : O_SH * (c + 1)] = yc
    return out.reshape(B, S, O), res


def kernel(x, qweight, lut, rows, cols, vals, bias):
    out, _ = _run(dict(x=x, qweight=qweight, lut=lut, rows=rows,
                       cols=cols, vals=vals, bias=bias))
    return out
